# revision 4
# baseline (speedup 1.0000x reference)
"""GAT (3-layer, PyG GATConv-style) Trainium2 Bass kernel, 8-core SPMD.

Instruction-count-minimized redesign (the axon path serializes at ~60us per
instruction, so wall time ~ total instruction count):

  - Nodes are sorted by in-degree and dealt round-robin to the 8 cores, so
    all cores share one compile-time block structure with near-identical
    per-slot degrees. dst-sharded edge parallelism as before.
  - Per layer, each core computes h_aug rows [h(H*C) | s_src(H)] for its own
    nodes (2 matmuls / 128 nodes), plus a transposed s_dst table [H, slots]
    (5 matmuls), AllGathers the row table to a full DRAM table, then
    aggregates per dst block entirely with wide vector ops:
      * transposed dma_gather pulls the src rows feature-transposed:
        g1[p, k, g, e] = row[idx[e]][128*g + p]  (<=896 idxs per gather)
      * scores e = s_src + s_dst, Prelu(0.2), Exp run on H partitions for a
        whole block per instruction; segment-sum over the fixed per-block
        degree D happens in a single strided tensor_reduce
      * alpha = (p/H) / denom is broadcast to 128 partitions (one SBUF DMA +
        one partition_broadcast), multiplied into the gathered features
        in-place, and a single 5-dim reduce produces the head-averaged
        numerator [C, nd] directly in next-layer-transposed layout.
  - Padding edges point at a sentinel row with s_src = -1e30 (p = 0);
    degree-0 pad slots get one edge to an all-zero neutral row (p > 0,
    h = 0) so denominators stay positive.
  - Layer 3 accumulates the node-sum per block; host sums cores' partials,
    divides by N and adds b3.
"""

import numpy as np
import ml_dtypes

BF16 = ml_dtypes.bfloat16
NCORES = 8

# chunk (= one multi-packet transposed gather) is Dc*nd idxs: <= 2688, %128
def _menu():
    out = []
    for nd in (64, 96, 128, 160, 192, 224, 256, 320, 384, 448, 512):
        for dc in range(2688 // nd, 0, -1):
            if (dc * nd) % 128 == 0:
                out.append((nd, dc))
                break
    return out
MENU = _menu()
CAPE = 5376          # max edges (idx slots) per block, SBUF-driven
SENT_OFF = 0         # sentinel row = NPAD + 0
NEUT_OFF = 1         # neutral row  = NPAD + 1


def _wrap16(idx_flat):
    n = idx_flat.shape[0]
    assert n % 16 == 0
    w = idx_flat.reshape(n // 16, 16).T.astype(np.int16)
    return np.tile(w, (8, 1))


def prep_static(edge_index, N):
    """Degree-sorted node permutation + shared block structure + per-core
    gather index tables."""
    E0 = edge_index.shape[1]
    loops = np.arange(N, dtype=np.int64)
    src = np.concatenate([edge_index[0].astype(np.int64), loops])
    dst = np.concatenate([edge_index[1].astype(np.int64), loops])
    deg = np.bincount(dst, minlength=N)

    order = np.argsort(-deg, kind="stable")
    node_core = np.empty(N, dtype=np.int64)
    node_slot = np.empty(N, dtype=np.int64)
    node_core[order] = np.arange(N) % NCORES
    node_slot[order] = np.arange(N) // NCORES
    NSLOT = (N + NCORES - 1) // NCORES          # 2500
    NPC = ((NSLOT + 127) // 128) * 128          # 2560 slots incl pads
    NPAD = NPC * NCORES

    # per (core, slot) degree; Dmax over cores per slot
    slotdeg = np.zeros((NCORES, NPC), dtype=np.int64)
    slotdeg[node_core, node_slot] = deg
    Dmax = slotdeg.max(axis=0)

    # greedy shared block structure over slots
    blocks = []
    s = 0
    while s < NSLOT:
        Dneed = max(int(Dmax[s]), 1)
        pick = None
        for nd, Dc in sorted(MENU, reverse=True):   # largest nd first
            if nd > NPC - s:
                continue
            K = max(1, -(-Dneed // Dc))
            if K * Dc * nd <= CAPE:
                pick = (nd, Dc, K)
                break
        assert pick is not None, f"no block fits at slot {s}"
        blocks.append((s,) + pick)
        s += pick[0]
    BLKS = tuple(blocks)

    # hf row of original node j (allgather is chunked by CR rows: chunk ck
    # holds rank c's rows [ck*CR,(ck+1)*CR) at ck*CR*NCORES + c*CR + r%CR)
    CR = 512
    hfrow = (node_slot // CR) * (CR * NCORES) + node_core * CR +         (node_slot % CR)

    # per-core per-slot src lists (ordered by slot)
    ecore = node_core[dst]
    eslot = node_slot[dst]
    eorder = np.argsort(ecore * NPC + eslot, kind="stable")
    src_s = src[eorder]
    key_s = (ecore * NPC + eslot)[eorder]
    bounds = np.searchsorted(key_s, np.arange(NCORES * NPC + 1))

    SENT = NPAD + SENT_OFF
    NEUT = NPAD + NEUT_OFF
    idx_cores = []
    for c in range(NCORES):
        cols = []
        for (s0, nd, Dc, K) in BLKS:
            tab = np.full((K * Dc, nd), SENT, dtype=np.int64)
            for n in range(nd):
                g = c * NPC + s0 + n
                lo, hi = int(bounds[g]), int(bounds[g + 1])
                dn = hi - lo
                if dn == 0:
                    tab[0, n] = NEUT
                else:
                    tab[:dn, n] = hfrow[src_s[lo:hi]]
            for k in range(K):
                cols.append(_wrap16(tab[k * Dc:(k + 1) * Dc].reshape(-1)))
        idx_cores.append(np.concatenate(cols, axis=1))
    return BLKS, idx_cores, node_core, node_slot, NPC, NPAD


def prep_values(x, Ws, a_srcs, a_dsts, node_core, node_slot, NPC):
    N, F = x.shape
    xT_cores = np.zeros((NCORES, F, NPC), dtype=np.float32)
    xT_cores[node_core, :, node_slot] = x          # fancy: [N, F] into [c][:,s]
    xT_cores = xT_cores.astype(BF16)

    W_augs, wdsts = [], []
    for W, a_s, a_d in zip(Ws, a_srcs, a_dsts):
        H, Fin, C = W.shape
        RW = H * C + 128
        Wf = np.transpose(W, (1, 0, 2)).reshape(Fin, H * C)
        wsrc = np.einsum("hfc,hc->fh", W, a_s)
        wdst = np.einsum("hfc,hc->fh", W, a_d)
        Wa = np.zeros((Fin, RW), dtype=np.float32)
        Wa[:, :H * C] = Wf
        Wa[:, H * C:H * C + H] = wsrc
        W_augs.append(Wa.astype(BF16))
        wdsts.append(wdst.astype(BF16))
    return xT_cores, W_augs, wdsts


# ----------------------------------------------------------------------------
# Device program
# ----------------------------------------------------------------------------

def build_nc(cfg, repeat=1):
    import concourse.bacc as bacc
    import concourse.mybir as mybir
    import concourse.tile as tile
    from contextlib import ExitStack

    f32 = mybir.dt.float32
    bf16 = mybir.dt.bfloat16
    i16 = mybir.dt.int16
    ALU = mybir.AluOpType
    ACT = mybir.ActivationFunctionType
    AX = mybir.AxisListType

    N = cfg["N"]
    NPC = cfg["NPC"]
    NPAD = NPC * NCORES
    F_IN = cfg["F_IN"]
    C = cfg["C"]
    HS = cfg["HS"]
    BLKS = cfg["BLKS"]
    NB = len(BLKS)
    NL = len(HS)
    RWs = [HS[i] * C + 128 for i in range(NL)]
    FINs = [F_IN] + [C] * (NL - 1)
    SUMI = sum(K * Dc * nd for (_, nd, Dc, K) in BLKS)
    NBA = NPC // 128                      # phase-A 128-node blocks

    nc = bacc.Bacc("TRN2", target_bir_lowering=False, debug=False,
                   num_devices=NCORES)

    xT_d = nc.dram_tensor("xT", [F_IN, NPC], bf16, kind="ExternalInput")
    idx_d = nc.dram_tensor("idx", [128, SUMI // 16], i16, kind="ExternalInput")
    W_d = [nc.dram_tensor(f"w{i+1}", [FINs[i], RWs[i]], bf16,
                          kind="ExternalInput") for i in range(NL)]
    wd_d = [nc.dram_tensor(f"wd{i+1}", [FINs[i], HS[i]], bf16,
                           kind="ExternalInput") for i in range(NL)]
    bb_d = [nc.dram_tensor(f"bb{i+1}", [C, 1], f32, kind="ExternalInput")
            for i in range(NL - 1)]
    out_d = nc.dram_tensor("out", [C, 1], f32, kind="ExternalOutput")

    with tile.TileContext(nc, num_cores=NCORES) as tc, ExitStack() as ctx:
        dram = ctx.enter_context(tc.tile_pool(name="dram", bufs=1, space="DRAM"))
        cpool = ctx.enter_context(tc.tile_pool(name="consts", bufs=1))
        hpool = ctx.enter_context(tc.tile_pool(name="haug", bufs=1))
        gpool = ctx.enter_context(tc.tile_pool(name="gath", bufs=1))
        wpool = ctx.enter_context(tc.tile_pool(name="work", bufs=1))
        apool = ctx.enter_context(tc.tile_pool(name="alpha", bufs=1))
        fpool = ctx.enter_context(tc.tile_pool(name="fin", bufs=1))
        psum = ctx.enter_context(tc.tile_pool(name="ps", bufs=2, space="PSUM"))

        hl = [dram.tile([NPC, RWs[i]], bf16, tag=f"hl{i}", name=f"hl{i}")
              for i in range(NL)]
        hf = [dram.tile([NPAD + 128, RWs[i]], bf16, tag=f"hf{i}",
                        name=f"hf{i}") for i in range(NL)]

        # ---- constants ----
        xT_sb = cpool.tile([F_IN, NPC], bf16, tag="xT")
        nc.sync.dma_start(xT_sb[:], xT_d[:, :])
        idx_sb = cpool.tile([128, SUMI // 16], i16, tag="idx")
        nc.sync.dma_start(idx_sb[:], idx_d[:, :])
        W_sb, wd_sb, bb_sb = [], [], []
        for i in range(NL):
            w = cpool.tile([FINs[i], RWs[i]], bf16, tag=f"w{i}", name=f"w{i}")
            nc.sync.dma_start(w[:], W_d[i][:, :])
            W_sb.append(w)
            wd = cpool.tile([FINs[i], HS[i]], bf16, tag=f"wd{i}", name=f"wd{i}")
            nc.sync.dma_start(wd[:], wd_d[i][:, :])
            wd_sb.append(wd)
        for i in range(NL - 1):
            b = cpool.tile([C, 1], f32, tag=f"bb{i}", name=f"bb{i}")
            nc.sync.dma_start(b[:], bb_d[i][:, :])
            bb_sb.append(b)

        # sentinel (s_src = -1e30) + neutral (all zero) rows per layer table
        for L in range(NL):
            HC = HS[L] * C
            srow = cpool.tile([1, 2, RWs[L]], bf16, tag=f"sr{L}", name=f"sr{L}")
            nc.vector.memset(srow[:], 0.0)
            nc.vector.memset(srow[:, 0, HC:HC + HS[L]], -1e30)
            nc.sync.dma_start(hf[L][NPAD:NPAD + 2, :], srow[:])

        # next-layer transposed features (phase C writes, phase A reads)
        x2T = [cpool.tile([C, NPC], bf16, tag=f"x2T{i}", name=f"x2T{i}")
               for i in range(NL - 1)]
        for t in x2T:
            nc.vector.memset(t[:], 0.0)

        nout = cpool.tile([C, NPC], f32, tag="nout")
        nc.vector.memset(nout[:], 0.0)

        # hoisted num_idxs registers (avoid one RegisterMove per gather)
        cnds = sorted({Dc * nd for (_, nd, Dc, K) in BLKS})
        cnd_reg = {v: nc.gpsimd.to_reg(v) for v in cnds}

        for _rep in range(repeat):
            for L in range(NL):
                H = HS[L]
                RW = RWs[L]
                G = RW // 128
                HC = H * C
                xin = xT_sb if L == 0 else x2T[L - 1]

                # ---- phase A: h_aug rows for own slots ----
                for nb0 in range(0, NBA, 4):
                    nb1 = min(nb0 + 4, NBA)
                    nw = nb1 - nb0
                    hs = hpool.tile([128, 4, RW], bf16, tag="hs")
                    for nb in range(nb0, nb1):
                        lhs = xin[:, nb * 128:(nb + 1) * 128]
                        j = nb - nb0
                        if RW > 512:
                            p1 = psum.tile([128, 640], f32, tag="pA")
                            nc.tensor.matmul(p1[:, 0:512], lhs, W_sb[L][:, 0:512],
                                             start=True, stop=True)
                            nc.tensor.matmul(p1[:, 512:RW], lhs, W_sb[L][:, 512:RW],
                                             start=True, stop=True)
                            nc.scalar.copy(hs[:, j, 0:RW], p1[:, 0:RW])
                        else:
                            p1 = psum.tile([128, RW], f32, tag="pA")
                            nc.tensor.matmul(p1[:], lhs, W_sb[L][:, 0:RW],
                                             start=True, stop=True)
                            nc.scalar.copy(hs[:, j, 0:RW], p1[:])
                    # hl[nb0*128 + j*128 + p] = hs[p, j, :]
                    orows = hl[L][nb0 * 128:nb1 * 128, :].rearrange(
                        "(j p) w -> p j w", j=nw)
                    nc.sync.dma_start(orows, hs[:, 0:nw, :])

                # ---- transposed s_dst for own slots: [H, NPC] ----
                sdT = cpool.tile([HS[L], NPC], bf16, tag=f"sdT{L}",
                                 name=f"sdT{L}")
                for j in range(NPC // 512):
                    ps = psum.tile([HS[L], 512], f32, tag="pS")
                    nc.tensor.matmul(ps[:], wd_sb[L],
                                     xin[:, j * 512:(j + 1) * 512],
                                     start=True, stop=True)
                    nc.scalar.copy(sdT[:, j * 512:(j + 1) * 512], ps[:])

                # ---- allgather ----
                CR = 512
                for ck in range(NPC // CR):
                    nc.gpsimd.collective_compute(
                        "AllGather", mybir.AluOpType.bypass,
                        replica_groups=[list(range(NCORES))],
                        ins=[hl[L][ck * CR:(ck + 1) * CR, :].opt()],
                        outs=[hf[L][ck * CR * NCORES:(ck + 1) * CR * NCORES,
                                    :].opt()],
                    )

                # ---- phase C ----
                col = 0
                for bi, (s0, nd, Dc, K) in enumerate(BLKS):
                    CND = Dc * nd
                    g1 = gpool.tile([128, K, G, CND], bf16, tag="g1")
                    for k in range(K):
                        nc.gpsimd.dma_gather(
                            g1[:, k, :, :], hf[L][:, :],
                            idx_sb[:, col:col + CND // 16],
                            CND, cnd_reg[CND], RW, transpose=True,
                            single_packet=False)
                        col += CND // 16

                    # scores -> p (in place), on H partitions
                    e = wpool.tile([H, K, Dc, nd], f32, tag="e")
                    ssrc = g1[0:H, :, G - 1, :].rearrange(
                        "p k (d n) -> p k d n", d=Dc)
                    sd_v = sdT[:, s0:s0 + nd].unsqueeze(1).unsqueeze(1) \
                        .broadcast_to((H, K, Dc, nd))
                    nc.vector.tensor_tensor(e[:], ssrc, sd_v, ALU.add)
                    e_3 = e[:].rearrange("p k d n -> p (k d) n")
                    nc.vector.scalar_tensor_tensor(e_3, e_3, 0.2, e_3,
                                                   op0=ALU.mult, op1=ALU.max)
                    nc.scalar.activation(e[:], e[:], ACT.Exp)

                    den = wpool.tile([H, nd], f32, tag="den")
                    nc.vector.tensor_reduce(den[:], e[:].transpose([0, 3, 1, 2]),
                                            AX.XY, ALU.add)
                    rc = wpool.tile([H, nd], f32, tag="rc")
                    nc.vector.reciprocal(rc[:], den[:])
                    al = wpool.tile([H, K, Dc, nd], bf16, tag="al")
                    rc_v = rc[:].unsqueeze(1).broadcast_to((H, K * Dc, nd))
                    e_f = e[:].rearrange("p k d n -> p (k d) n")
                    al_f = al[:].rearrange("p k d n -> p (k d) n")
                    nc.vector.scalar_tensor_tensor(al_f, e_f, 1.0 / H, rc_v,
                                                   op0=ALU.mult, op1=ALU.mult)

                    # broadcast alpha to all 128 partitions
                    aa = apool.tile([128, H, K * CND], bf16, tag="aa")
                    if H > 1:
                        adr = dram.tile([H, K * CND], bf16, tag="adr",
                                        name="adr")
                        nc.gpsimd.dma_start(adr[:], al[:])
                        nc.gpsimd.dma_start(
                            aa[:], adr[:].unsqueeze(0).broadcast_to(
                                (128, H, K * CND)))
                    else:
                        nc.gpsimd.partition_broadcast(aa[:], al[0:1, :, :, :])

                    # msg = h * alpha, in place on gathered feature groups
                    g1f = g1[:, :, 0:H, :]
                    aa_v = aa[:].rearrange("p h (k c) -> p k h c", k=K)
                    nc.vector.tensor_tensor(g1f, g1f, aa_v, ALU.mult)

                    # numer + head mean: [128, nd] in transposed layout
                    if L < NL - 1:
                        num_t = fpool.tile([C, nd], f32, tag="num", name="num_t")
                        num_ap = num_t[:]
                    else:
                        num_ap = nout[:, s0:s0 + nd]
                    if H > 1:
                        nv = g1[:, :, 0:H, :].rearrange(
                            "p k h (d n) -> p n h k d", d=Dc)
                        nc.vector.tensor_reduce(num_ap, nv, AX.XYZ, ALU.add)
                    else:
                        nv = g1[:, :, 0, :].rearrange(
                            "p k (d n) -> p n k d", d=Dc)
                        nc.vector.tensor_reduce(num_ap, nv, AX.XY, ALU.add)

                    if L < NL - 1:
                        nc.vector.tensor_scalar(x2T[L][:, s0:s0 + nd], num_ap,
                                                bb_sb[L][:], 0.0,
                                                op0=ALU.add, op1=ALU.max)

        fo = fpool.tile([C, 1], f32, tag="fo")
        nc.vector.tensor_reduce(fo[:], nout[:], AX.X, ALU.add)
        nc.sync.dma_start(out_d[:, :], fo[:])

    nc.compile()
    return nc


# ----------------------------------------------------------------------------
# Entry points
# ----------------------------------------------------------------------------

def make_cfg_and_maps(inputs):
    x = np.asarray(inputs["x"])
    edge_index = np.asarray(inputs["edge_index"])
    N, F_IN = x.shape
    Ws = [np.asarray(inputs[f"W{i}"]) for i in (1, 2, 3)]
    a_srcs = [np.asarray(inputs[f"as{i}"]) for i in (1, 2, 3)]
    a_dsts = [np.asarray(inputs[f"ad{i}"]) for i in (1, 2, 3)]
    bs = [np.asarray(inputs[f"b{i}"]) for i in (1, 2, 3)]
    HS = tuple(W.shape[0] for W in Ws)
    C = Ws[0].shape[2]

    BLKS, idx_cores, node_core, node_slot, NPC, NPAD = \
        prep_static(edge_index, N)
    xT_cores, W_augs, wdsts = prep_values(
        x, Ws, a_srcs, a_dsts, node_core, node_slot, NPC)

    cfg = dict(N=N, NPC=NPC, F_IN=F_IN, C=C, HS=HS, BLKS=BLKS)
    in_maps = []
    for c in range(NCORES):
        m = {
            "xT": np.ascontiguousarray(xT_cores[c]),
            "idx": idx_cores[c],
        }
        for i in range(3):
            m[f"w{i+1}"] = W_augs[i]
            m[f"wd{i+1}"] = wdsts[i]
        for i in range(2):
            m[f"bb{i+1}"] = bs[i].astype(np.float32).reshape(C, 1)
        in_maps.append(m)
    return cfg, in_maps, bs[2]


_NC_CACHE = {}


def _get_nc(cfg, repeat=1):
    key = (repeat, cfg["N"], cfg["NPC"], cfg["F_IN"], cfg["C"], cfg["HS"],
           cfg["BLKS"])
    if key not in _NC_CACHE:
        _NC_CACHE[key] = build_nc(cfg, repeat=repeat)
    return _NC_CACHE[key]


def run(inputs, trace=False, repeat=1, **kw):
    from concourse.bass_utils import run_bass_kernel_spmd
    cfg, in_maps, b3 = make_cfg_and_maps(inputs)
    nc = _get_nc(cfg, repeat=repeat)
    res = run_bass_kernel_spmd(nc, in_maps, core_ids=list(range(NCORES)),
                               trace=trace, **kw)
    acc = np.zeros((cfg["C"],), dtype=np.float32)
    for r in res.results:
        acc += r["out"].reshape(-1)
    out = (acc / cfg["N"] + b3.astype(np.float32)).reshape(1, cfg["C"])
    return out, res


def kernel(**inputs) -> np.ndarray:
    out, _ = run(inputs)
    return out


# revision 5
# speedup vs baseline: 1.0219x; 1.0219x over previous
"""GAT (3-layer, PyG GATConv-style) Trainium2 Bass kernel, 8-core SPMD.

Instruction-count-minimized redesign (the axon path serializes at ~60us per
instruction, so wall time ~ total instruction count):

  - Nodes are sorted by in-degree and dealt round-robin to the 8 cores, so
    all cores share one compile-time block structure with near-identical
    per-slot degrees. dst-sharded edge parallelism as before.
  - Per layer, each core computes h_aug rows [h(H*C) | s_src(H)] for its own
    nodes (2 matmuls / 128 nodes), plus a transposed s_dst table [H, slots]
    (5 matmuls), AllGathers the row table to a full DRAM table, then
    aggregates per dst block entirely with wide vector ops:
      * transposed dma_gather pulls the src rows feature-transposed:
        g1[p, k, g, e] = row[idx[e]][128*g + p]  (<=896 idxs per gather)
      * scores e = s_src + s_dst, Prelu(0.2), Exp run on H partitions for a
        whole block per instruction; segment-sum over the fixed per-block
        degree D happens in a single strided tensor_reduce
      * alpha = (p/H) / denom is broadcast to 128 partitions (one SBUF DMA +
        one partition_broadcast), multiplied into the gathered features
        in-place, and a single 5-dim reduce produces the head-averaged
        numerator [C, nd] directly in next-layer-transposed layout.
  - Padding edges point at a sentinel row with s_src = -1e30 (p = 0);
    degree-0 pad slots get one edge to an all-zero neutral row (p > 0,
    h = 0) so denominators stay positive.
  - Layer 3 accumulates the node-sum per block; host sums cores' partials,
    divides by N and adds b3.
"""

import numpy as np
import ml_dtypes

BF16 = ml_dtypes.bfloat16
NCORES = 8

# chunk (= one multi-packet transposed gather) is Dc*nd idxs: <= 2688, %128
def _menu():
    out = []
    for nd in (64, 96, 128, 160, 192, 224, 256, 320, 384, 448, 512):
        for dc in range(2688 // nd, 0, -1):
            if (dc * nd) % 128 == 0:
                out.append((nd, dc))
                break
    return out
MENU = _menu()
CAPE = 5376          # max edges (idx slots) per block, SBUF-driven
SENT_OFF = 0         # sentinel row = NPAD + 0
NEUT_OFF = 1         # neutral row  = NPAD + 1


def _wrap16(idx_flat):
    n = idx_flat.shape[0]
    assert n % 16 == 0
    w = idx_flat.reshape(n // 16, 16).T.astype(np.int16)
    return np.tile(w, (8, 1))


def prep_static(edge_index, N):
    """Degree-sorted node permutation + shared block structure + per-core
    gather index tables."""
    E0 = edge_index.shape[1]
    loops = np.arange(N, dtype=np.int64)
    src = np.concatenate([edge_index[0].astype(np.int64), loops])
    dst = np.concatenate([edge_index[1].astype(np.int64), loops])
    deg = np.bincount(dst, minlength=N)

    order = np.argsort(-deg, kind="stable")
    node_core = np.empty(N, dtype=np.int64)
    node_slot = np.empty(N, dtype=np.int64)
    node_core[order] = np.arange(N) % NCORES
    node_slot[order] = np.arange(N) // NCORES
    NSLOT = (N + NCORES - 1) // NCORES          # 2500
    NPC = ((NSLOT + 127) // 128) * 128          # 2560 slots incl pads
    NPAD = NPC * NCORES

    # per (core, slot) degree; Dmax over cores per slot
    slotdeg = np.zeros((NCORES, NPC), dtype=np.int64)
    slotdeg[node_core, node_slot] = deg
    Dmax = slotdeg.max(axis=0)

    # greedy shared block structure over slots
    blocks = []
    s = 0
    while s < NSLOT:
        Dneed = max(int(Dmax[s]), 1)
        pick = None
        for nd, Dc in sorted(MENU, reverse=True):   # largest nd first
            if nd > NPC - s:
                continue
            K = max(1, -(-Dneed // Dc))
            if K * Dc * nd <= CAPE:
                pick = (nd, Dc, K)
                break
        assert pick is not None, f"no block fits at slot {s}"
        blocks.append((s,) + pick)
        s += pick[0]
    BLKS = tuple(blocks)

    # hf row of original node j (allgather is chunked by CR rows: chunk ck
    # holds rank c's rows [ck*CR,(ck+1)*CR) at ck*CR*NCORES + c*CR + r%CR)
    CR = 512
    hfrow = (node_slot // CR) * (CR * NCORES) + node_core * CR +         (node_slot % CR)

    # per-core per-slot src lists (ordered by slot)
    ecore = node_core[dst]
    eslot = node_slot[dst]
    eorder = np.argsort(ecore * NPC + eslot, kind="stable")
    src_s = src[eorder]
    key_s = (ecore * NPC + eslot)[eorder]
    bounds = np.searchsorted(key_s, np.arange(NCORES * NPC + 1))

    SENT = NPAD + SENT_OFF
    NEUT = NPAD + NEUT_OFF
    idx_cores = []
    for c in range(NCORES):
        cols = []
        for (s0, nd, Dc, K) in BLKS:
            tab = np.full((K * Dc, nd), SENT, dtype=np.int64)
            for n in range(nd):
                g = c * NPC + s0 + n
                lo, hi = int(bounds[g]), int(bounds[g + 1])
                dn = hi - lo
                if dn == 0:
                    tab[0, n] = NEUT
                else:
                    tab[:dn, n] = hfrow[src_s[lo:hi]]
            for k in range(K):
                cols.append(_wrap16(tab[k * Dc:(k + 1) * Dc].reshape(-1)))
        idx_cores.append(np.concatenate(cols, axis=1))
    return BLKS, idx_cores, node_core, node_slot, NPC, NPAD


def prep_values(x, Ws, a_srcs, a_dsts, node_core, node_slot, NPC):
    N, F = x.shape
    xT_cores = np.zeros((NCORES, F, NPC), dtype=np.float32)
    xT_cores[node_core, :, node_slot] = x          # fancy: [N, F] into [c][:,s]
    xT_cores = xT_cores.astype(BF16)

    W_augs, wdsts = [], []
    for W, a_s, a_d in zip(Ws, a_srcs, a_dsts):
        H, Fin, C = W.shape
        RW = H * C + 128
        Wf = np.transpose(W, (1, 0, 2)).reshape(Fin, H * C)
        wsrc = np.einsum("hfc,hc->fh", W, a_s)
        wdst = np.einsum("hfc,hc->fh", W, a_d)
        Wa = np.zeros((Fin, RW), dtype=np.float32)
        Wa[:, :H * C] = Wf
        Wa[:, H * C:H * C + H] = wsrc
        W_augs.append(Wa.astype(BF16))
        wdsts.append(wdst.astype(BF16))
    return xT_cores, W_augs, wdsts


# ----------------------------------------------------------------------------
# Device program
# ----------------------------------------------------------------------------

def build_nc(cfg, repeat=1):
    import concourse.bacc as bacc
    import concourse.mybir as mybir
    import concourse.tile as tile
    from contextlib import ExitStack

    f32 = mybir.dt.float32
    bf16 = mybir.dt.bfloat16
    i16 = mybir.dt.int16
    ALU = mybir.AluOpType
    ACT = mybir.ActivationFunctionType
    AX = mybir.AxisListType

    N = cfg["N"]
    NPC = cfg["NPC"]
    NPAD = NPC * NCORES
    F_IN = cfg["F_IN"]
    C = cfg["C"]
    HS = cfg["HS"]
    BLKS = cfg["BLKS"]
    NB = len(BLKS)
    NL = len(HS)
    RWs = [HS[i] * C + 128 for i in range(NL)]
    FINs = [F_IN] + [C] * (NL - 1)
    SUMI = sum(K * Dc * nd for (_, nd, Dc, K) in BLKS)
    NBA = NPC // 128                      # phase-A 128-node blocks

    nc = bacc.Bacc("TRN2", target_bir_lowering=False, debug=False,
                   num_devices=NCORES)

    xT_d = nc.dram_tensor("xT", [F_IN, NPC], bf16, kind="ExternalInput")
    idx_d = nc.dram_tensor("idx", [128, SUMI // 16], i16, kind="ExternalInput")
    W_d = [nc.dram_tensor(f"w{i+1}", [FINs[i], RWs[i]], bf16,
                          kind="ExternalInput") for i in range(NL)]
    wd_d = [nc.dram_tensor(f"wd{i+1}", [FINs[i], HS[i]], bf16,
                           kind="ExternalInput") for i in range(NL)]
    bb_d = [nc.dram_tensor(f"bb{i+1}", [C, 1], f32, kind="ExternalInput")
            for i in range(NL - 1)]
    out_d = nc.dram_tensor("out", [C, 1], f32, kind="ExternalOutput")

    with tile.TileContext(nc, num_cores=NCORES) as tc, ExitStack() as ctx:
        dram = ctx.enter_context(tc.tile_pool(name="dram", bufs=1, space="DRAM"))
        cpool = ctx.enter_context(tc.tile_pool(name="consts", bufs=1))
        hpool = ctx.enter_context(tc.tile_pool(name="haug", bufs=1))
        gpool = ctx.enter_context(tc.tile_pool(name="gath", bufs=1))
        wpool = ctx.enter_context(tc.tile_pool(name="work", bufs=1))
        apool = ctx.enter_context(tc.tile_pool(name="alpha", bufs=1))
        fpool = ctx.enter_context(tc.tile_pool(name="fin", bufs=1))
        psum = ctx.enter_context(tc.tile_pool(name="ps", bufs=2, space="PSUM"))

        hl = [dram.tile([NPC, RWs[i]], bf16, tag=f"hl{i}", name=f"hl{i}")
              for i in range(NL)]
        hf = [dram.tile([NPAD + 128, RWs[i]], bf16, tag=f"hf{i}",
                        name=f"hf{i}") for i in range(NL)]

        # ---- constants ----
        xT_sb = cpool.tile([F_IN, NPC], bf16, tag="xT")
        nc.sync.dma_start(xT_sb[:], xT_d[:, :])
        idx_sb = cpool.tile([128, SUMI // 16], i16, tag="idx")
        nc.sync.dma_start(idx_sb[:], idx_d[:, :])
        W_sb, wd_sb, bb_sb = [], [], []
        for i in range(NL):
            w = cpool.tile([FINs[i], RWs[i]], bf16, tag=f"w{i}", name=f"w{i}")
            nc.sync.dma_start(w[:], W_d[i][:, :])
            W_sb.append(w)
            wd = cpool.tile([FINs[i], HS[i]], bf16, tag=f"wd{i}", name=f"wd{i}")
            nc.sync.dma_start(wd[:], wd_d[i][:, :])
            wd_sb.append(wd)
        for i in range(NL - 1):
            b = cpool.tile([C, 1], f32, tag=f"bb{i}", name=f"bb{i}")
            nc.sync.dma_start(b[:], bb_d[i][:, :])
            bb_sb.append(b)

        # sentinel (s_src = -1e30) + neutral (all zero) rows per layer table
        for L in range(NL):
            HC = HS[L] * C
            srow = cpool.tile([1, 2, RWs[L]], bf16, tag=f"sr{L}", name=f"sr{L}")
            nc.vector.memset(srow[:], 0.0)
            nc.vector.memset(srow[:, 0, HC:HC + HS[L]], -1e30)
            nc.sync.dma_start(hf[L][NPAD:NPAD + 2, :], srow[:])

        # next-layer transposed features (phase C writes, phase A reads)
        x2T = [cpool.tile([C, NPC], bf16, tag=f"x2T{i}", name=f"x2T{i}")
               for i in range(NL - 1)]
        for t in x2T:
            nc.vector.memset(t[:], 0.0)

        nout = cpool.tile([C, NPC], f32, tag="nout")
        nc.vector.memset(nout[:], 0.0)

        # hoisted num_idxs registers (avoid one RegisterMove per gather)
        cnds = sorted({Dc * nd for (_, nd, Dc, K) in BLKS})
        cnd_reg = {v: nc.gpsimd.to_reg(v) for v in cnds}

        for _rep in range(repeat):
            for L in range(NL):
                H = HS[L]
                RW = RWs[L]
                G = RW // 128
                HC = H * C
                xin = xT_sb if L == 0 else x2T[L - 1]

                # ---- phase A: h_aug rows for own slots ----
                HB = NBA // 2
                for h0 in (0, HB):
                 hs = hpool.tile([128, HB, RW], bf16, tag="hs", name="hs")
                 for nb in range(h0, h0 + HB):
                    lhs = xin[:, nb * 128:(nb + 1) * 128]
                    if RW > 512:
                        p1 = psum.tile([128, 640], f32, tag="pA")
                        nc.tensor.matmul(p1[:, 0:512], lhs, W_sb[L][:, 0:512],
                                         start=True, stop=True)
                        nc.tensor.matmul(p1[:, 512:RW], lhs, W_sb[L][:, 512:RW],
                                         start=True, stop=True)
                        nc.scalar.copy(hs[:, nb - h0, 0:RW], p1[:, 0:RW])
                    elif nb % 2 == 0:
                        # pair two 256-wide blocks in one PSUM bank + one copy
                        p1 = psum.tile([128, 512], f32, tag="pA")
                        nc.tensor.matmul(p1[:, 0:RW], lhs, W_sb[L][:, 0:RW],
                                         start=True, stop=True)
                        nc.tensor.matmul(p1[:, RW:2 * RW],
                                         xin[:, (nb + 1) * 128:(nb + 2) * 128],
                                         W_sb[L][:, 0:RW],
                                         start=True, stop=True)
                        nc.scalar.copy(hs[:, nb - h0:nb - h0 + 2, :], p1[:])
                 orows = hl[L][h0 * 128:(h0 + HB) * 128, :].rearrange(
                     "(j p) w -> p j w", j=HB)
                 nc.sync.dma_start(orows, hs[:])

                # ---- transposed s_dst for own slots: [H, NPC] ----
                sdT = cpool.tile([HS[L], NPC], bf16, tag=f"sdT{L}",
                                 name=f"sdT{L}")
                for j in range(NPC // 512):
                    ps = psum.tile([HS[L], 512], f32, tag="pS")
                    nc.tensor.matmul(ps[:], wd_sb[L],
                                     xin[:, j * 512:(j + 1) * 512],
                                     start=True, stop=True)
                    nc.scalar.copy(sdT[:, j * 512:(j + 1) * 512], ps[:])

                # ---- allgather ----
                CR = 512
                for ck in range(NPC // CR):
                    nc.gpsimd.collective_compute(
                        "AllGather", mybir.AluOpType.bypass,
                        replica_groups=[list(range(NCORES))],
                        ins=[hl[L][ck * CR:(ck + 1) * CR, :].opt()],
                        outs=[hf[L][ck * CR * NCORES:(ck + 1) * CR * NCORES,
                                    :].opt()],
                    )

                # ---- phase C ----
                col = 0
                for bi, (s0, nd, Dc, K) in enumerate(BLKS):
                    CND = Dc * nd
                    g1 = gpool.tile([128, K, G, CND], bf16, tag="g1")
                    for k in range(K):
                        nc.gpsimd.dma_gather(
                            g1[:, k, :, :], hf[L][:, :],
                            idx_sb[:, col:col + CND // 16],
                            CND, cnd_reg[CND], RW, transpose=True,
                            single_packet=False)
                        col += CND // 16

                    # scores -> p (in place), on H partitions
                    e = wpool.tile([H, K, Dc, nd], f32, tag="e")
                    ssrc = g1[0:H, :, G - 1, :].rearrange(
                        "p k (d n) -> p k d n", d=Dc)
                    sd_v = sdT[:, s0:s0 + nd].unsqueeze(1).unsqueeze(1) \
                        .broadcast_to((H, K, Dc, nd))
                    nc.vector.tensor_tensor(e[:], ssrc, sd_v, ALU.add)
                    e_3 = e[:].rearrange("p k d n -> p (k d) n")
                    nc.vector.scalar_tensor_tensor(e_3, e_3, 0.2, e_3,
                                                   op0=ALU.mult, op1=ALU.max)
                    nc.scalar.activation(e[:], e[:], ACT.Exp)

                    den = wpool.tile([H, nd], f32, tag="den")
                    nc.vector.tensor_reduce(den[:], e[:].transpose([0, 3, 1, 2]),
                                            AX.XY, ALU.add)
                    rc = wpool.tile([H, nd], f32, tag="rc")
                    nc.vector.reciprocal(rc[:], den[:])
                    al = wpool.tile([H, K, Dc, nd], bf16, tag="al")
                    rc_v = rc[:].unsqueeze(1).broadcast_to((H, K * Dc, nd))
                    e_f = e[:].rearrange("p k d n -> p (k d) n")
                    al_f = al[:].rearrange("p k d n -> p (k d) n")
                    nc.vector.scalar_tensor_tensor(al_f, e_f, 1.0 / H, rc_v,
                                                   op0=ALU.mult, op1=ALU.mult)

                    # broadcast alpha to all 128 partitions
                    aa = apool.tile([128, H, K * CND], bf16, tag="aa")
                    if H > 1:
                        adr = dram.tile([H, K * CND], bf16, tag="adr",
                                        name="adr")
                        nc.gpsimd.dma_start(adr[:], al[:])
                        nc.gpsimd.dma_start(
                            aa[:], adr[:].unsqueeze(0).broadcast_to(
                                (128, H, K * CND)))
                    else:
                        nc.gpsimd.partition_broadcast(aa[:], al[0:1, :, :, :])

                    # msg = h * alpha, in place on gathered feature groups
                    g1f = g1[:, :, 0:H, :]
                    aa_v = aa[:].rearrange("p h (k c) -> p k h c", k=K)
                    nc.vector.tensor_tensor(g1f, g1f, aa_v, ALU.mult)

                    # numer + head mean: [128, nd] in transposed layout
                    if L < NL - 1:
                        num_t = fpool.tile([C, nd], f32, tag="num", name="num_t")
                        num_ap = num_t[:]
                    else:
                        num_ap = nout[:, s0:s0 + nd]
                    if H > 1:
                        nv = g1[:, :, 0:H, :].rearrange(
                            "p k h (d n) -> p n h k d", d=Dc)
                        nc.vector.tensor_reduce(num_ap, nv, AX.XYZ, ALU.add)
                    else:
                        nv = g1[:, :, 0, :].rearrange(
                            "p k (d n) -> p n k d", d=Dc)
                        nc.vector.tensor_reduce(num_ap, nv, AX.XY, ALU.add)

                    if L < NL - 1:
                        nc.vector.tensor_scalar(x2T[L][:, s0:s0 + nd], num_ap,
                                                bb_sb[L][:], 0.0,
                                                op0=ALU.add, op1=ALU.max)

        fo = fpool.tile([C, 1], f32, tag="fo")
        nc.vector.tensor_reduce(fo[:], nout[:], AX.X, ALU.add)
        nc.sync.dma_start(out_d[:, :], fo[:])

    nc.compile()
    return nc


# ----------------------------------------------------------------------------
# Entry points
# ----------------------------------------------------------------------------

def make_cfg_and_maps(inputs):
    x = np.asarray(inputs["x"])
    edge_index = np.asarray(inputs["edge_index"])
    N, F_IN = x.shape
    Ws = [np.asarray(inputs[f"W{i}"]) for i in (1, 2, 3)]
    a_srcs = [np.asarray(inputs[f"as{i}"]) for i in (1, 2, 3)]
    a_dsts = [np.asarray(inputs[f"ad{i}"]) for i in (1, 2, 3)]
    bs = [np.asarray(inputs[f"b{i}"]) for i in (1, 2, 3)]
    HS = tuple(W.shape[0] for W in Ws)
    C = Ws[0].shape[2]

    BLKS, idx_cores, node_core, node_slot, NPC, NPAD = \
        prep_static(edge_index, N)
    xT_cores, W_augs, wdsts = prep_values(
        x, Ws, a_srcs, a_dsts, node_core, node_slot, NPC)

    cfg = dict(N=N, NPC=NPC, F_IN=F_IN, C=C, HS=HS, BLKS=BLKS)
    in_maps = []
    for c in range(NCORES):
        m = {
            "xT": np.ascontiguousarray(xT_cores[c]),
            "idx": idx_cores[c],
        }
        for i in range(3):
            m[f"w{i+1}"] = W_augs[i]
            m[f"wd{i+1}"] = wdsts[i]
        for i in range(2):
            m[f"bb{i+1}"] = bs[i].astype(np.float32).reshape(C, 1)
        in_maps.append(m)
    return cfg, in_maps, bs[2]


_NC_CACHE = {}


def _get_nc(cfg, repeat=1):
    key = (repeat, cfg["N"], cfg["NPC"], cfg["F_IN"], cfg["C"], cfg["HS"],
           cfg["BLKS"])
    if key not in _NC_CACHE:
        _NC_CACHE[key] = build_nc(cfg, repeat=repeat)
    return _NC_CACHE[key]


def run(inputs, trace=False, repeat=1, **kw):
    from concourse.bass_utils import run_bass_kernel_spmd
    cfg, in_maps, b3 = make_cfg_and_maps(inputs)
    nc = _get_nc(cfg, repeat=repeat)
    res = run_bass_kernel_spmd(nc, in_maps, core_ids=list(range(NCORES)),
                               trace=trace, **kw)
    acc = np.zeros((cfg["C"],), dtype=np.float32)
    for r in res.results:
        acc += r["out"].reshape(-1)
    out = (acc / cfg["N"] + b3.astype(np.float32)).reshape(1, cfg["C"])
    return out, res


def kernel(**inputs) -> np.ndarray:
    out, _ = run(inputs)
    return out


# revision 6
# speedup vs baseline: 1.0544x; 1.0318x over previous
"""GAT (3-layer, PyG GATConv-style) Trainium2 Bass kernel, 8-core SPMD.

Instruction-count-minimized redesign (the axon path serializes at ~60us per
instruction, so wall time ~ total instruction count):

  - Nodes are sorted by in-degree and dealt round-robin to the 8 cores, so
    all cores share one compile-time block structure with near-identical
    per-slot degrees. dst-sharded edge parallelism as before.
  - Per layer, each core computes h_aug rows [h(H*C) | s_src(H)] for its own
    nodes (2 matmuls / 128 nodes), plus a transposed s_dst table [H, slots]
    (5 matmuls), AllGathers the row table to a full DRAM table, then
    aggregates per dst block entirely with wide vector ops:
      * transposed dma_gather pulls the src rows feature-transposed:
        g1[p, k, g, e] = row[idx[e]][128*g + p]  (<=896 idxs per gather)
      * scores e = s_src + s_dst, Prelu(0.2), Exp run on H partitions for a
        whole block per instruction; segment-sum over the fixed per-block
        degree D happens in a single strided tensor_reduce
      * alpha = (p/H) / denom is broadcast to 128 partitions (one SBUF DMA +
        one partition_broadcast), multiplied into the gathered features
        in-place, and a single 5-dim reduce produces the head-averaged
        numerator [C, nd] directly in next-layer-transposed layout.
  - Padding edges point at a sentinel row with s_src = -1e30 (p = 0);
    degree-0 pad slots get one edge to an all-zero neutral row (p > 0,
    h = 0) so denominators stay positive.
  - Layer 3 accumulates the node-sum per block; host sums cores' partials,
    divides by N and adds b3.
"""

import numpy as np
import ml_dtypes

BF16 = ml_dtypes.bfloat16
NCORES = 8

# chunk (= one multi-packet transposed gather) is Dc*nd idxs: <= 2688, %128
def _menu():
    out = []
    for nd in (64, 96, 128, 160, 192, 224, 256, 320, 384, 448, 512):
        for dc in range(2688 // nd, 0, -1):
            if (dc * nd) % 128 == 0:
                out.append((nd, dc))
                break
    return out
MENU = _menu()
CAPE = 5376          # max edges (idx slots) per block, SBUF-driven
SENT_OFF = 0         # sentinel row = NPAD + 0
NEUT_OFF = 1         # neutral row  = NPAD + 1


def _wrap16(idx_flat):
    n = idx_flat.shape[0]
    assert n % 16 == 0
    w = idx_flat.reshape(n // 16, 16).T.astype(np.int16)
    return np.tile(w, (8, 1))


def prep_static(edge_index, N):
    """Degree-sorted node permutation + shared block structure + per-core
    gather index tables."""
    E0 = edge_index.shape[1]
    loops = np.arange(N, dtype=np.int64)
    src = np.concatenate([edge_index[0].astype(np.int64), loops])
    dst = np.concatenate([edge_index[1].astype(np.int64), loops])
    deg = np.bincount(dst, minlength=N)

    order = np.argsort(-deg, kind="stable")
    node_core = np.empty(N, dtype=np.int64)
    node_slot = np.empty(N, dtype=np.int64)
    node_core[order] = np.arange(N) % NCORES
    node_slot[order] = np.arange(N) // NCORES
    NSLOT = (N + NCORES - 1) // NCORES          # 2500
    NPC = ((NSLOT + 127) // 128) * 128          # 2560 slots incl pads
    NPAD = NPC * NCORES

    # per (core, slot) degree; Dmax over cores per slot
    slotdeg = np.zeros((NCORES, NPC), dtype=np.int64)
    slotdeg[node_core, node_slot] = deg
    Dmax = slotdeg.max(axis=0)

    # greedy shared block structure over slots
    blocks = []
    s = 0
    while s < NSLOT:
        Dneed = max(int(Dmax[s]), 1)
        pick = None
        for nd, Dc in sorted(MENU, reverse=True):   # largest nd first
            if nd > NPC - s:
                continue
            K = max(1, -(-Dneed // Dc))
            if K * Dc * nd <= CAPE:
                pick = (nd, Dc, K)
                break
        assert pick is not None, f"no block fits at slot {s}"
        blocks.append((s,) + pick)
        s += pick[0]
    BLKS = tuple(blocks)

    # hf row of original node j (allgather is chunked by CR rows: chunk ck
    # holds rank c's rows [ck*CR,(ck+1)*CR) at ck*CR*NCORES + c*CR + r%CR)
    CR = 512
    hfrow = (node_slot // CR) * (CR * NCORES) + node_core * CR +         (node_slot % CR)

    # per-core per-slot src lists (ordered by slot)
    ecore = node_core[dst]
    eslot = node_slot[dst]
    eorder = np.argsort(ecore * NPC + eslot, kind="stable")
    src_s = src[eorder]
    key_s = (ecore * NPC + eslot)[eorder]
    bounds = np.searchsorted(key_s, np.arange(NCORES * NPC + 1))

    SENT = NPAD + SENT_OFF
    NEUT = NPAD + NEUT_OFF
    idx_cores = []
    for c in range(NCORES):
        cols = []
        for (s0, nd, Dc, K) in BLKS:
            tab = np.full((K * Dc, nd), SENT, dtype=np.int64)
            for n in range(nd):
                g = c * NPC + s0 + n
                lo, hi = int(bounds[g]), int(bounds[g + 1])
                dn = hi - lo
                if dn == 0:
                    tab[0, n] = NEUT
                else:
                    tab[:dn, n] = hfrow[src_s[lo:hi]]
            for k in range(K):
                cols.append(_wrap16(tab[k * Dc:(k + 1) * Dc].reshape(-1)))
        idx_cores.append(np.concatenate(cols, axis=1))
    return BLKS, idx_cores, node_core, node_slot, NPC, NPAD


def prep_values(x, Ws, a_srcs, a_dsts, node_core, node_slot, NPC):
    N, F = x.shape
    xT_cores = np.zeros((NCORES, F, NPC), dtype=np.float32)
    xT_cores[node_core, :, node_slot] = x          # fancy: [N, F] into [c][:,s]
    xT_cores = xT_cores.astype(BF16)

    W_augs, wdsts = [], []
    for W, a_s, a_d in zip(Ws, a_srcs, a_dsts):
        H, Fin, C = W.shape
        RW = H * C + 128
        Wf = np.transpose(W, (1, 0, 2)).reshape(Fin, H * C)
        wsrc = np.einsum("hfc,hc->fh", W, a_s)
        wdst = np.einsum("hfc,hc->fh", W, a_d)
        Wa = np.zeros((Fin, RW), dtype=np.float32)
        Wa[:, :H * C] = Wf
        Wa[:, H * C:H * C + H] = wsrc
        W_augs.append(Wa.astype(BF16))
        wdsts.append(wdst.astype(BF16))
    return xT_cores, W_augs, wdsts


# ----------------------------------------------------------------------------
# Device program
# ----------------------------------------------------------------------------

def build_nc(cfg, repeat=1):
    import concourse.bacc as bacc
    import concourse.mybir as mybir
    import concourse.tile as tile
    from contextlib import ExitStack

    f32 = mybir.dt.float32
    bf16 = mybir.dt.bfloat16
    i16 = mybir.dt.int16
    ALU = mybir.AluOpType
    ACT = mybir.ActivationFunctionType
    AX = mybir.AxisListType

    N = cfg["N"]
    NPC = cfg["NPC"]
    NPAD = NPC * NCORES
    F_IN = cfg["F_IN"]
    C = cfg["C"]
    HS = cfg["HS"]
    BLKS = cfg["BLKS"]
    NB = len(BLKS)
    NL = len(HS)
    RWs = [HS[i] * C + 128 for i in range(NL)]
    FINs = [F_IN] + [C] * (NL - 1)
    SUMI = sum(K * Dc * nd for (_, nd, Dc, K) in BLKS)
    NBA = NPC // 128                      # phase-A 128-node blocks

    nc = bacc.Bacc("TRN2", target_bir_lowering=False, debug=False,
                   num_devices=NCORES)

    xT_d = nc.dram_tensor("xT", [F_IN, NPC], bf16, kind="ExternalInput")
    idx_d = nc.dram_tensor("idx", [128, SUMI // 16], i16, kind="ExternalInput")
    W_d = [nc.dram_tensor(f"w{i+1}", [FINs[i], RWs[i]], bf16,
                          kind="ExternalInput") for i in range(NL)]
    wd_d = [nc.dram_tensor(f"wd{i+1}", [FINs[i], HS[i]], bf16,
                           kind="ExternalInput") for i in range(NL)]
    bb_d = [nc.dram_tensor(f"bb{i+1}", [C, 1], f32, kind="ExternalInput")
            for i in range(NL - 1)]
    out_d = nc.dram_tensor("out", [C, 1], f32, kind="ExternalOutput")

    with tile.TileContext(nc, num_cores=NCORES) as tc, ExitStack() as ctx:
        dram = ctx.enter_context(tc.tile_pool(name="dram", bufs=1, space="DRAM"))
        cpool = ctx.enter_context(tc.tile_pool(name="consts", bufs=1))
        hpool = ctx.enter_context(tc.tile_pool(name="haug", bufs=1))
        gpool = ctx.enter_context(tc.tile_pool(name="gath", bufs=1))
        wpool = ctx.enter_context(tc.tile_pool(name="work", bufs=1))
        apool = ctx.enter_context(tc.tile_pool(name="alpha", bufs=1))
        fpool = ctx.enter_context(tc.tile_pool(name="fin", bufs=1))
        psum = ctx.enter_context(tc.tile_pool(name="ps", bufs=2, space="PSUM"))

        hl = [dram.tile([NPC, RWs[i]], bf16, tag=f"hl{i}", name=f"hl{i}")
              for i in range(NL)]
        hf = [dram.tile([NPAD + 128, RWs[i]], bf16, tag=f"hf{i}",
                        name=f"hf{i}") for i in range(NL)]

        # ---- constants ----
        xT_sb = cpool.tile([F_IN, NPC], bf16, tag="xT")
        nc.sync.dma_start(xT_sb[:], xT_d[:, :])
        idx_sb = cpool.tile([128, SUMI // 16], i16, tag="idx")
        nc.sync.dma_start(idx_sb[:], idx_d[:, :])
        W_sb, wd_sb, bb_sb = [], [], []
        for i in range(NL):
            w = cpool.tile([FINs[i], RWs[i]], bf16, tag=f"w{i}", name=f"w{i}")
            nc.sync.dma_start(w[:], W_d[i][:, :])
            W_sb.append(w)
            wd = cpool.tile([FINs[i], HS[i]], bf16, tag=f"wd{i}", name=f"wd{i}")
            nc.sync.dma_start(wd[:], wd_d[i][:, :])
            wd_sb.append(wd)
        for i in range(NL - 1):
            b = cpool.tile([C, 1], f32, tag=f"bb{i}", name=f"bb{i}")
            nc.sync.dma_start(b[:], bb_d[i][:, :])
            bb_sb.append(b)

        # sentinel (s_src = -1e30) + neutral (all zero) rows per layer table
        for L in range(NL):
            HC = HS[L] * C
            srow = hpool.tile([1, 2, RWs[L]], bf16, tag="srow", name="srow")
            nc.vector.memset(srow[:], 0.0)
            nc.vector.memset(srow[:, 0, HC:HC + HS[L]], -1e30)
            nc.sync.dma_start(hf[L][NPAD:NPAD + 2, :], srow[:])

        # next-layer transposed features (phase C writes, phase A reads)
        x2T = [cpool.tile([C, NPC], bf16, tag=f"x2T{i}", name=f"x2T{i}")
               for i in range(NL - 1)]
        for t in x2T:
            nc.vector.memset(t[:], 0.0)

        nout = cpool.tile([C, NPC], f32, tag="nout")
        nc.vector.memset(nout[:], 0.0)

        # hoisted num_idxs registers (avoid one RegisterMove per gather)
        cnds = sorted({Dc * nd for (_, nd, Dc, K) in BLKS})
        cnd_reg = {v: nc.gpsimd.to_reg(v) for v in cnds}

        for _rep in range(repeat):
            for L in range(NL):
                H = HS[L]
                RW = RWs[L]
                G = RW // 128
                HC = H * C
                xin = xT_sb if L == 0 else x2T[L - 1]

                # ---- phase A: h_aug rows for own slots ----
                HB = NBA // 2
                for h0 in (0, HB):
                 hs = hpool.tile([128, HB, RW], bf16, tag="hs", name="hs")
                 for nb in range(h0, h0 + HB):
                    lhs = xin[:, nb * 128:(nb + 1) * 128]
                    if RW > 512:
                        p1 = psum.tile([128, 640], f32, tag="pA")
                        nc.tensor.matmul(p1[:, 0:512], lhs, W_sb[L][:, 0:512],
                                         start=True, stop=True)
                        nc.tensor.matmul(p1[:, 512:RW], lhs, W_sb[L][:, 512:RW],
                                         start=True, stop=True)
                        nc.scalar.copy(hs[:, nb - h0, 0:RW], p1[:, 0:RW])
                    elif nb % 2 == 0:
                        # pair two 256-wide blocks in one PSUM bank + one copy
                        p1 = psum.tile([128, 512], f32, tag="pA")
                        nc.tensor.matmul(p1[:, 0:RW], lhs, W_sb[L][:, 0:RW],
                                         start=True, stop=True)
                        nc.tensor.matmul(p1[:, RW:2 * RW],
                                         xin[:, (nb + 1) * 128:(nb + 2) * 128],
                                         W_sb[L][:, 0:RW],
                                         start=True, stop=True)
                        nc.scalar.copy(hs[:, nb - h0:nb - h0 + 2, :], p1[:])
                 orows = hl[L][h0 * 128:(h0 + HB) * 128, :].rearrange(
                     "(j p) w -> p j w", j=HB)
                 nc.sync.dma_start(orows, hs[:])

                # ---- transposed s_dst for own slots: [H, NPC] ----
                sdT = wpool.tile([HS[L], NPC], bf16, tag="sdT", name="sdT")
                for j in range(NPC // 512):
                    ps = psum.tile([HS[L], 512], f32, tag="pS")
                    nc.tensor.matmul(ps[:], wd_sb[L],
                                     xin[:, j * 512:(j + 1) * 512],
                                     start=True, stop=True)
                    nc.scalar.copy(sdT[:, j * 512:(j + 1) * 512], ps[:])

                # ---- allgather ----
                CR = 512
                for ck in range(NPC // CR):
                    nc.gpsimd.collective_compute(
                        "AllGather", mybir.AluOpType.bypass,
                        replica_groups=[list(range(NCORES))],
                        ins=[hl[L][ck * CR:(ck + 1) * CR, :].opt()],
                        outs=[hf[L][ck * CR * NCORES:(ck + 1) * CR * NCORES,
                                    :].opt()],
                    )

                # ---- phase C ----
                col = 0
                for bi, (s0, nd, Dc, K) in enumerate(BLKS):
                    CND = Dc * nd
                    g1 = gpool.tile([128, K, G, CND], bf16, tag="g1")
                    for k in range(K):
                        nc.gpsimd.dma_gather(
                            g1[:, k, :, :], hf[L][:, :],
                            idx_sb[:, col:col + CND // 16],
                            CND, cnd_reg[CND], RW, transpose=True,
                            single_packet=False)
                        col += CND // 16

                    # scores -> p (in place), on H partitions
                    e = wpool.tile([H, K, Dc, nd], f32, tag="e")
                    ssrc = g1[0:H, :, G - 1, :].rearrange(
                        "p k (d n) -> p k d n", d=Dc)
                    sd_v = sdT[:, s0:s0 + nd].unsqueeze(1).unsqueeze(1) \
                        .broadcast_to((H, K, Dc, nd))
                    nc.vector.tensor_tensor(e[:], ssrc, sd_v, ALU.add)
                    e_3 = e[:].rearrange("p k d n -> p (k d) n")
                    nc.vector.scalar_tensor_tensor(e_3, e_3, 0.2, e_3,
                                                   op0=ALU.mult, op1=ALU.max)
                    nc.scalar.activation(e[:], e[:], ACT.Exp)

                    den = wpool.tile([H, nd], f32, tag="den")
                    nc.vector.tensor_reduce(den[:], e[:].transpose([0, 3, 1, 2]),
                                            AX.XY, ALU.add)
                    rc = wpool.tile([H, nd], f32, tag="rc")
                    nc.vector.reciprocal(rc[:], den[:])
                    al = wpool.tile([H, K, Dc, nd], bf16, tag="al")
                    rc_v = rc[:].unsqueeze(1).broadcast_to((H, K * Dc, nd))
                    e_f = e[:].rearrange("p k d n -> p (k d) n")
                    al_f = al[:].rearrange("p k d n -> p (k d) n")
                    nc.vector.scalar_tensor_tensor(al_f, e_f, 1.0 / H, rc_v,
                                                   op0=ALU.mult, op1=ALU.mult)

                    # broadcast alpha to all 128 partitions
                    aa = apool.tile([128, H, K * CND], bf16, tag="aa")
                    if H > 1:
                        adr = dram.tile([H, K * CND], bf16, tag="adr",
                                        name="adr")
                        nc.gpsimd.dma_start(adr[:], al[:])
                        nc.gpsimd.dma_start(
                            aa[:], adr[:].unsqueeze(0).broadcast_to(
                                (128, H, K * CND)))
                    else:
                        nc.gpsimd.partition_broadcast(aa[:], al[0:1, :, :, :])

                    # msg = h * alpha, in place on gathered feature groups
                    g1f = g1[:, :, 0:H, :]
                    aa_v = aa[:].rearrange("p h (k c) -> p k h c", k=K)
                    nc.vector.tensor_tensor(g1f, g1f, aa_v, ALU.mult)

                    # numer + head mean: [128, nd] in transposed layout
                    if L < NL - 1:
                        num_t = fpool.tile([C, nd], f32, tag="num", name="num_t")
                        num_ap = num_t[:]
                    else:
                        num_ap = nout[:, s0:s0 + nd]
                    if H > 1:
                        nv = g1[:, :, 0:H, :].rearrange(
                            "p k h (d n) -> p n h k d", d=Dc)
                        nc.vector.tensor_reduce(num_ap, nv, AX.XYZ, ALU.add)
                    else:
                        nv = g1[:, :, 0, :].rearrange(
                            "p k (d n) -> p n k d", d=Dc)
                        nc.vector.tensor_reduce(num_ap, nv, AX.XY, ALU.add)

                    if L < NL - 1:
                        nc.vector.tensor_scalar(x2T[L][:, s0:s0 + nd], num_ap,
                                                bb_sb[L][:], 0.0,
                                                op0=ALU.add, op1=ALU.max)

        fo = fpool.tile([C, 1], f32, tag="fo")
        nc.vector.tensor_reduce(fo[:], nout[:], AX.X, ALU.add)
        nc.sync.dma_start(out_d[:, :], fo[:])

    nc.compile()
    return nc


# ----------------------------------------------------------------------------
# Entry points
# ----------------------------------------------------------------------------

def make_cfg_and_maps(inputs):
    x = np.asarray(inputs["x"])
    edge_index = np.asarray(inputs["edge_index"])
    N, F_IN = x.shape
    Ws = [np.asarray(inputs[f"W{i}"]) for i in (1, 2, 3)]
    a_srcs = [np.asarray(inputs[f"as{i}"]) for i in (1, 2, 3)]
    a_dsts = [np.asarray(inputs[f"ad{i}"]) for i in (1, 2, 3)]
    bs = [np.asarray(inputs[f"b{i}"]) for i in (1, 2, 3)]
    HS = tuple(W.shape[0] for W in Ws)
    C = Ws[0].shape[2]

    BLKS, idx_cores, node_core, node_slot, NPC, NPAD = \
        prep_static(edge_index, N)
    xT_cores, W_augs, wdsts = prep_values(
        x, Ws, a_srcs, a_dsts, node_core, node_slot, NPC)

    cfg = dict(N=N, NPC=NPC, F_IN=F_IN, C=C, HS=HS, BLKS=BLKS)
    in_maps = []
    for c in range(NCORES):
        m = {
            "xT": np.ascontiguousarray(xT_cores[c]),
            "idx": idx_cores[c],
        }
        for i in range(3):
            m[f"w{i+1}"] = W_augs[i]
            m[f"wd{i+1}"] = wdsts[i]
        for i in range(2):
            m[f"bb{i+1}"] = bs[i].astype(np.float32).reshape(C, 1)
        in_maps.append(m)
    return cfg, in_maps, bs[2]


_NC_CACHE = {}


def _get_nc(cfg, repeat=1):
    key = (repeat, cfg["N"], cfg["NPC"], cfg["F_IN"], cfg["C"], cfg["HS"],
           cfg["BLKS"])
    if key not in _NC_CACHE:
        _NC_CACHE[key] = build_nc(cfg, repeat=repeat)
    return _NC_CACHE[key]


def run(inputs, trace=False, repeat=1, **kw):
    from concourse.bass_utils import run_bass_kernel_spmd
    cfg, in_maps, b3 = make_cfg_and_maps(inputs)
    nc = _get_nc(cfg, repeat=repeat)
    res = run_bass_kernel_spmd(nc, in_maps, core_ids=list(range(NCORES)),
                               trace=trace, **kw)
    acc = np.zeros((cfg["C"],), dtype=np.float32)
    for r in res.results:
        acc += r["out"].reshape(-1)
    out = (acc / cfg["N"] + b3.astype(np.float32)).reshape(1, cfg["C"])
    return out, res


def kernel(**inputs) -> np.ndarray:
    out, _ = run(inputs)
    return out


# revision 7
# speedup vs baseline: 1.1210x; 1.0632x over previous
"""GAT (3-layer, PyG GATConv-style) Trainium2 Bass kernel, 8-core SPMD.

Instruction-count-minimized redesign (the axon path serializes at ~60us per
instruction, so wall time ~ total instruction count):

  - Nodes are sorted by in-degree and dealt round-robin to the 8 cores, so
    all cores share one compile-time block structure with near-identical
    per-slot degrees. dst-sharded edge parallelism as before.
  - Per layer, each core computes h_aug rows [h(H*C) | s_src(H)] for its own
    nodes (2 matmuls / 128 nodes), plus a transposed s_dst table [H, slots]
    (5 matmuls), AllGathers the row table to a full DRAM table, then
    aggregates per dst block entirely with wide vector ops:
      * transposed dma_gather pulls the src rows feature-transposed:
        g1[p, k, g, e] = row[idx[e]][128*g + p]  (<=896 idxs per gather)
      * scores e = s_src + s_dst, Prelu(0.2), Exp run on H partitions for a
        whole block per instruction; segment-sum over the fixed per-block
        degree D happens in a single strided tensor_reduce
      * alpha = (p/H) / denom is broadcast to 128 partitions (one SBUF DMA +
        one partition_broadcast), multiplied into the gathered features
        in-place, and a single 5-dim reduce produces the head-averaged
        numerator [C, nd] directly in next-layer-transposed layout.
  - Padding edges point at a sentinel row with s_src = -1e30 (p = 0);
    degree-0 pad slots get one edge to an all-zero neutral row (p > 0,
    h = 0) so denominators stay positive.
  - Layer 3 accumulates the node-sum per block; host sums cores' partials,
    divides by N and adds b3.
"""

import numpy as np
import ml_dtypes

BF16 = ml_dtypes.bfloat16
NCORES = 8

# chunk (= one multi-packet transposed gather) is Dc*nd idxs: <= 2688, %128
def _menu():
    out = []
    for nd in (64, 96, 128, 160, 192, 224, 256, 320, 384, 448, 512):
        for dc in range(2688 // nd, 0, -1):
            if (dc * nd) % 128 == 0:
                out.append((nd, dc))
                break
    return out
MENU = _menu()
CAPE = 5376          # max edges (idx slots) per block, SBUF-driven
SENT_OFF = 0         # sentinel row = NPAD + 0
NEUT_OFF = 1         # neutral row  = NPAD + 1


def _wrap16(idx_flat):
    n = idx_flat.shape[0]
    assert n % 16 == 0
    w = idx_flat.reshape(n // 16, 16).T.astype(np.int16)
    return np.tile(w, (8, 1))


def prep_static(edge_index, N):
    """Degree-sorted node permutation + shared block structure + per-core
    gather index tables."""
    E0 = edge_index.shape[1]
    loops = np.arange(N, dtype=np.int64)
    src = np.concatenate([edge_index[0].astype(np.int64), loops])
    dst = np.concatenate([edge_index[1].astype(np.int64), loops])
    deg = np.bincount(dst, minlength=N)

    order = np.argsort(-deg, kind="stable")
    node_core = np.empty(N, dtype=np.int64)
    node_slot = np.empty(N, dtype=np.int64)
    node_core[order] = np.arange(N) % NCORES
    node_slot[order] = np.arange(N) // NCORES
    NSLOT = (N + NCORES - 1) // NCORES          # 2500
    NPC = ((NSLOT + 127) // 128) * 128          # 2560 slots incl pads
    NPAD = NPC * NCORES

    # per (core, slot) degree; Dmax over cores per slot
    slotdeg = np.zeros((NCORES, NPC), dtype=np.int64)
    slotdeg[node_core, node_slot] = deg
    Dmax = slotdeg.max(axis=0)

    # greedy shared block structure over slots
    blocks = []
    s = 0
    while s < NSLOT:
        Dneed = max(int(Dmax[s]), 1)
        pick = None
        for nd, Dc in sorted(MENU, reverse=True):   # largest nd first
            if nd > NPC - s:
                continue
            K = max(1, -(-Dneed // Dc))
            if K * Dc * nd <= CAPE:
                pick = (nd, Dc, K)
                break
        assert pick is not None, f"no block fits at slot {s}"
        blocks.append((s,) + pick)
        s += pick[0]
    BLKS = tuple(blocks)

    # hf row of original node j (allgather is chunked by CR rows: chunk ck
    # holds rank c's rows [ck*CR,(ck+1)*CR) at ck*CR*NCORES + c*CR + r%CR)
    CR = 512
    hfrow = (node_slot // CR) * (CR * NCORES) + node_core * CR +         (node_slot % CR)

    # per-core per-slot src lists (ordered by slot)
    ecore = node_core[dst]
    eslot = node_slot[dst]
    eorder = np.argsort(ecore * NPC + eslot, kind="stable")
    src_s = src[eorder]
    key_s = (ecore * NPC + eslot)[eorder]
    bounds = np.searchsorted(key_s, np.arange(NCORES * NPC + 1))

    SENT = NPAD + SENT_OFF
    NEUT = NPAD + NEUT_OFF
    idx_cores = []
    for c in range(NCORES):
        cols = []
        for (s0, nd, Dc, K) in BLKS:
            tab = np.full((K * Dc, nd), SENT, dtype=np.int64)
            for n in range(nd):
                g = c * NPC + s0 + n
                lo, hi = int(bounds[g]), int(bounds[g + 1])
                dn = hi - lo
                if dn == 0:
                    tab[0, n] = NEUT
                else:
                    tab[:dn, n] = hfrow[src_s[lo:hi]]
            for k in range(K):
                cols.append(_wrap16(tab[k * Dc:(k + 1) * Dc].reshape(-1)))
        idx_cores.append(np.concatenate(cols, axis=1))
    return BLKS, idx_cores, node_core, node_slot, NPC, NPAD


def prep_values(x, Ws, a_srcs, a_dsts, node_core, node_slot, NPC):
    N, F = x.shape
    xT_cores = np.zeros((NCORES, F, NPC), dtype=np.float32)
    xT_cores[node_core, :, node_slot] = x          # fancy: [N, F] into [c][:,s]
    xT_cores = xT_cores.astype(BF16)

    W_augs, wdsts = [], []
    for W, a_s, a_d in zip(Ws, a_srcs, a_dsts):
        H, Fin, C = W.shape
        RW = H * C + 128
        Wf = np.transpose(W, (1, 0, 2)).reshape(Fin, H * C)
        wsrc = np.einsum("hfc,hc->fh", W, a_s)
        wdst = np.einsum("hfc,hc->fh", W, a_d)
        Wa = np.zeros((Fin, RW), dtype=np.float32)
        Wa[:, :H * C] = Wf
        Wa[:, H * C:H * C + H] = wsrc
        W_augs.append(Wa.astype(BF16))
        wdsts.append(wdst.astype(BF16))
    return xT_cores, W_augs, wdsts


# ----------------------------------------------------------------------------
# Device program
# ----------------------------------------------------------------------------

def build_nc(cfg, repeat=1):
    import concourse.bacc as bacc
    import concourse.mybir as mybir
    import concourse.tile as tile
    from contextlib import ExitStack

    f32 = mybir.dt.float32
    bf16 = mybir.dt.bfloat16
    i16 = mybir.dt.int16
    ALU = mybir.AluOpType
    ACT = mybir.ActivationFunctionType
    AX = mybir.AxisListType

    N = cfg["N"]
    NPC = cfg["NPC"]
    NPAD = NPC * NCORES
    F_IN = cfg["F_IN"]
    C = cfg["C"]
    HS = cfg["HS"]
    BLKS = cfg["BLKS"]
    NB = len(BLKS)
    NL = len(HS)
    RWs = [HS[i] * C + 128 for i in range(NL)]
    FINs = [F_IN] + [C] * (NL - 1)
    SUMI = sum(K * Dc * nd for (_, nd, Dc, K) in BLKS)
    NBA = NPC // 128                      # phase-A 128-node blocks

    nc = bacc.Bacc("TRN2", target_bir_lowering=False, debug=False,
                   num_devices=NCORES)

    xT_d = nc.dram_tensor("xT", [F_IN, NPC], bf16, kind="ExternalInput")
    idx_d = nc.dram_tensor("idx", [128, SUMI // 16], i16, kind="ExternalInput")
    W_d = [nc.dram_tensor(f"w{i+1}", [FINs[i], RWs[i]], bf16,
                          kind="ExternalInput") for i in range(NL)]
    wd_d = [nc.dram_tensor(f"wd{i+1}", [FINs[i], HS[i]], bf16,
                           kind="ExternalInput") for i in range(NL)]
    bb_d = [nc.dram_tensor(f"bb{i+1}", [C, 1], f32, kind="ExternalInput")
            for i in range(NL - 1)]
    out_d = nc.dram_tensor("out", [C, 1], f32, kind="ExternalOutput")

    with tile.TileContext(nc, num_cores=NCORES) as tc, ExitStack() as ctx:
        dram = ctx.enter_context(tc.tile_pool(name="dram", bufs=1, space="DRAM"))
        cpool = ctx.enter_context(tc.tile_pool(name="consts", bufs=1))
        hpool = ctx.enter_context(tc.tile_pool(name="haug", bufs=1))
        gpool = ctx.enter_context(tc.tile_pool(name="gath", bufs=1))
        wpool = ctx.enter_context(tc.tile_pool(name="work", bufs=1))
        apool = ctx.enter_context(tc.tile_pool(name="alpha", bufs=1))
        fpool = ctx.enter_context(tc.tile_pool(name="fin", bufs=1))
        psum = ctx.enter_context(tc.tile_pool(name="ps", bufs=2, space="PSUM"))

        hl = [dram.tile([NPC, RWs[i]], bf16, tag=f"hl{i}", name=f"hl{i}")
              for i in range(NL)]
        hf = [dram.tile([NPAD + 128, RWs[i]], bf16, tag=f"hf{i}",
                        name=f"hf{i}") for i in range(NL)]

        # ---- constants ----
        xT_sb = cpool.tile([F_IN, NPC], bf16, tag="xT")
        nc.sync.dma_start(xT_sb[:], xT_d[:, :])
        idx_sb = cpool.tile([128, SUMI // 16], i16, tag="idx")
        nc.sync.dma_start(idx_sb[:], idx_d[:, :])
        W_sb, wd_sb, bb_sb = [], [], []
        for i in range(NL):
            w = cpool.tile([FINs[i], RWs[i]], bf16, tag=f"w{i}", name=f"w{i}")
            nc.sync.dma_start(w[:], W_d[i][:, :])
            W_sb.append(w)
            wd = cpool.tile([FINs[i], HS[i]], bf16, tag=f"wd{i}", name=f"wd{i}")
            nc.sync.dma_start(wd[:], wd_d[i][:, :])
            wd_sb.append(wd)
        for i in range(NL - 1):
            b = cpool.tile([C, 1], f32, tag=f"bb{i}", name=f"bb{i}")
            nc.sync.dma_start(b[:], bb_d[i][:, :])
            bb_sb.append(b)

        # sentinel (s_src = -1e30) + neutral (all zero) rows per layer table
        for L in range(NL):
            HC = HS[L] * C
            srow = hpool.tile([1, 2, RWs[L]], bf16, tag="srow", name="srow")
            nc.vector.memset(srow[:], 0.0)
            nc.vector.memset(srow[:, 0, HC:HC + HS[L]], -1e30)
            nc.sync.dma_start(hf[L][NPAD:NPAD + 2, :], srow[:])

        # next-layer transposed features (phase C writes, phase A reads)
        x2T = [cpool.tile([C, NPC], bf16, tag=f"x2T{i}", name=f"x2T{i}")
               for i in range(NL - 1)]
        for t in x2T:
            nc.vector.memset(t[:], 0.0)

        nout = cpool.tile([C, NPC], f32, tag="nout")
        nc.vector.memset(nout[:], 0.0)

        # hoisted num_idxs registers (avoid one RegisterMove per gather)
        cnds = sorted({Dc * nd for (_, nd, Dc, K) in BLKS})
        cnd_reg = {v: nc.gpsimd.to_reg(v) for v in cnds}

        for _rep in range(repeat):
            for L in range(NL):
                H = HS[L]
                RW = RWs[L]
                G = RW // 128
                HC = H * C
                xin = xT_sb if L == 0 else x2T[L - 1]

                # ---- phase A: h_aug rows for own slots ----
                HB = NBA // 2
                for h0 in (0, HB):
                 hs = hpool.tile([128, HB, RW], bf16, tag="hs", name="hs")
                 for nb in range(h0, h0 + HB):
                    lhs = xin[:, nb * 128:(nb + 1) * 128]
                    if RW > 512:
                        p1 = psum.tile([128, 640], f32, tag="pA")
                        nc.tensor.matmul(p1[:, 0:512], lhs, W_sb[L][:, 0:512],
                                         start=True, stop=True)
                        nc.tensor.matmul(p1[:, 512:RW], lhs, W_sb[L][:, 512:RW],
                                         start=True, stop=True)
                        nc.scalar.copy(hs[:, nb - h0, 0:RW], p1[:, 0:RW])
                    elif nb % 2 == 0:
                        # pair two 256-wide blocks in one PSUM bank + one copy
                        p1 = psum.tile([128, 512], f32, tag="pA")
                        nc.tensor.matmul(p1[:, 0:RW], lhs, W_sb[L][:, 0:RW],
                                         start=True, stop=True)
                        nc.tensor.matmul(p1[:, RW:2 * RW],
                                         xin[:, (nb + 1) * 128:(nb + 2) * 128],
                                         W_sb[L][:, 0:RW],
                                         start=True, stop=True)
                        nc.scalar.copy(hs[:, nb - h0:nb - h0 + 2, :], p1[:])
                 orows = hl[L][h0 * 128:(h0 + HB) * 128, :].rearrange(
                     "(j p) w -> p j w", j=HB)
                 nc.sync.dma_start(orows, hs[:])

                # ---- transposed s_dst for own slots: [H, NPC] ----
                sdT = wpool.tile([HS[L], NPC], bf16, tag="sdT", name="sdT")
                for j in range(NPC // 512):
                    ps = psum.tile([HS[L], 512], f32, tag="pS")
                    nc.tensor.matmul(ps[:], wd_sb[L],
                                     xin[:, j * 512:(j + 1) * 512],
                                     start=True, stop=True)
                    nc.scalar.copy(sdT[:, j * 512:(j + 1) * 512], ps[:])

                # ---- allgather ----
                CR = 512
                for ck in range(NPC // CR):
                    nc.gpsimd.collective_compute(
                        "AllGather", mybir.AluOpType.bypass,
                        replica_groups=[list(range(NCORES))],
                        ins=[hl[L][ck * CR:(ck + 1) * CR, :].opt()],
                        outs=[hf[L][ck * CR * NCORES:(ck + 1) * CR * NCORES,
                                    :].opt()],
                    )

                # ---- phase C ----
                col = 0
                bi = 0
                while bi < NB:
                    (s0, nd, Dc, K) = BLKS[bi]
                    CND = Dc * nd
                    # H==1: pair adjacent identical blocks (alpha needs no
                    # 1/H scale, so 4-dim tensor_tensor normalizes both)
                    M = 2 if (H == 1 and bi + 1 < NB and
                              BLKS[bi + 1] == (s0 + nd, nd, Dc, K)) else 1
                    MK = M * K
                    g1 = gpool.tile([128, MK, G, CND], bf16, tag="g1")
                    for k in range(MK):
                        nc.gpsimd.dma_gather(
                            g1[:, k, :, :], hf[L][:, :],
                            idx_sb[:, col:col + CND // 16],
                            CND, cnd_reg[CND], RW, transpose=True,
                            single_packet=False)
                        col += CND // 16

                    if H > 1:
                        # scores -> p (in place), on H partitions
                        e = wpool.tile([H, K, Dc, nd], f32, tag="e")
                        ssrc = g1[0:H, :, G - 1, :].rearrange(
                            "p k (d n) -> p k d n", d=Dc)
                        sd_v = sdT[:, s0:s0 + nd].unsqueeze(1).unsqueeze(1) \
                            .broadcast_to((H, K, Dc, nd))
                        nc.vector.tensor_tensor(e[:], ssrc, sd_v, ALU.add)
                        e_3 = e[:].rearrange("p k d n -> p (k d) n")
                        nc.vector.scalar_tensor_tensor(e_3, e_3, 0.2, e_3,
                                                       op0=ALU.mult,
                                                       op1=ALU.max)
                        nc.scalar.activation(e[:], e[:], ACT.Exp)

                        den = wpool.tile([H, nd], f32, tag="den")
                        nc.vector.tensor_reduce(den[:],
                                                e[:].transpose([0, 3, 1, 2]),
                                                AX.XY, ALU.add)
                        rc = wpool.tile([H, nd], f32, tag="rc")
                        nc.vector.reciprocal(rc[:], den[:])
                        al = wpool.tile([H, K, Dc, nd], bf16, tag="al")
                        rc_v = rc[:].unsqueeze(1).broadcast_to((H, K * Dc, nd))
                        e_f = e[:].rearrange("p k d n -> p (k d) n")
                        al_f = al[:].rearrange("p k d n -> p (k d) n")
                        nc.vector.scalar_tensor_tensor(al_f, e_f, 1.0 / H,
                                                       rc_v, op0=ALU.mult,
                                                       op1=ALU.mult)

                        aa = apool.tile([128, H, K * CND], bf16, tag="aa")
                        adr = dram.tile([H, K * CND], bf16, tag="adr",
                                        name="adr")
                        nc.gpsimd.dma_start(adr[:], al[:])
                        nc.gpsimd.dma_start(
                            aa[:], adr[:].unsqueeze(0).broadcast_to(
                                (128, H, K * CND)))

                        g1f = g1[:, :, 0:H, :]
                        aa_v = aa[:].rearrange("p h (k c) -> p k h c", k=K)
                        nc.vector.tensor_tensor(g1f, g1f, aa_v, ALU.mult)

                        num_t = fpool.tile([C, nd], f32, tag="num",
                                           name="num_t")
                        nv = g1[:, :, 0:H, :].rearrange(
                            "p k h (d n) -> p n h k d", d=Dc)
                        nc.vector.tensor_reduce(num_t[:], nv, AX.XYZ, ALU.add)
                        nc.vector.tensor_scalar(x2T[L][:, s0:s0 + nd],
                                                num_t[:], bb_sb[L][:], 0.0,
                                                op0=ALU.add, op1=ALU.max)
                    else:
                        # paired single-head path, scores in bf16
                        e = wpool.tile([1, MK, Dc, nd], bf16, tag="e",
                                       name="e3")
                        for m in range(M):
                            ssrc_m = g1[0:1, m * K:(m + 1) * K, G - 1, :] \
                                .rearrange("p k (d n) -> p k d n", d=Dc)
                            sd_m = sdT[0:1, s0 + m * nd:s0 + (m + 1) * nd] \
                                .unsqueeze(1).unsqueeze(1) \
                                .broadcast_to((1, K, Dc, nd))
                            nc.vector.tensor_tensor(
                                e[:, m * K:(m + 1) * K, :, :], ssrc_m, sd_m,
                                ALU.add)
                        e_3 = e[:].rearrange("p k d n -> p (k d) n")
                        nc.vector.scalar_tensor_tensor(e_3, e_3, 0.2, e_3,
                                                       op0=ALU.mult,
                                                       op1=ALU.max)
                        nc.scalar.activation(e[:], e[:], ACT.Exp)

                        den = wpool.tile([1, M, nd], f32, tag="den",
                                         name="den3")
                        dv = e[:].rearrange("p (m k) d n -> p m n (k d)", m=M)
                        nc.vector.tensor_reduce(den[:], dv, AX.X, ALU.add)
                        rc = wpool.tile([1, M, nd], f32, tag="rc", name="rc3")
                        nc.vector.reciprocal(rc[:], den[:])
                        al = wpool.tile([1, MK, Dc, nd], bf16, tag="al")
                        al_v = al[:].rearrange("p (m k) d n -> p m (k d) n",
                                               m=M)
                        e_v2 = e[:].rearrange("p (m k) d n -> p m (k d) n",
                                              m=M)
                        rc_v = rc[:].unsqueeze(2).broadcast_to(
                            (1, M, K * Dc, nd))
                        nc.vector.tensor_tensor(al_v, e_v2, rc_v, ALU.mult)

                        aa = apool.tile([128, MK * CND], bf16, tag="aa")
                        nc.gpsimd.partition_broadcast(aa[:],
                                                      al[0:1, :, :, :])
                        g1f = g1[:, :, 0, :]
                        aa_v = aa[:].rearrange("p (k c) -> p k c", k=MK)
                        nc.vector.tensor_tensor(g1f, g1f, aa_v, ALU.mult)

                        nv = g1[:, :, 0, :].rearrange(
                            "p (m k) (d n) -> p m n k d", m=M, d=Dc)
                        nc.vector.tensor_reduce(nout[:, s0:s0 + M * nd], nv,
                                                AX.XY, ALU.add)
                    bi += M

        fo = fpool.tile([C, 1], f32, tag="fo")
        nc.vector.tensor_reduce(fo[:], nout[:], AX.X, ALU.add)
        nc.sync.dma_start(out_d[:, :], fo[:])

    nc.compile()
    return nc


# ----------------------------------------------------------------------------
# Entry points
# ----------------------------------------------------------------------------

def make_cfg_and_maps(inputs):
    x = np.asarray(inputs["x"])
    edge_index = np.asarray(inputs["edge_index"])
    N, F_IN = x.shape
    Ws = [np.asarray(inputs[f"W{i}"]) for i in (1, 2, 3)]
    a_srcs = [np.asarray(inputs[f"as{i}"]) for i in (1, 2, 3)]
    a_dsts = [np.asarray(inputs[f"ad{i}"]) for i in (1, 2, 3)]
    bs = [np.asarray(inputs[f"b{i}"]) for i in (1, 2, 3)]
    HS = tuple(W.shape[0] for W in Ws)
    C = Ws[0].shape[2]

    BLKS, idx_cores, node_core, node_slot, NPC, NPAD = \
        prep_static(edge_index, N)
    xT_cores, W_augs, wdsts = prep_values(
        x, Ws, a_srcs, a_dsts, node_core, node_slot, NPC)

    cfg = dict(N=N, NPC=NPC, F_IN=F_IN, C=C, HS=HS, BLKS=BLKS)
    in_maps = []
    for c in range(NCORES):
        m = {
            "xT": np.ascontiguousarray(xT_cores[c]),
            "idx": idx_cores[c],
        }
        for i in range(3):
            m[f"w{i+1}"] = W_augs[i]
            m[f"wd{i+1}"] = wdsts[i]
        for i in range(2):
            m[f"bb{i+1}"] = bs[i].astype(np.float32).reshape(C, 1)
        in_maps.append(m)
    return cfg, in_maps, bs[2]


_NC_CACHE = {}


def _get_nc(cfg, repeat=1):
    key = (repeat, cfg["N"], cfg["NPC"], cfg["F_IN"], cfg["C"], cfg["HS"],
           cfg["BLKS"])
    if key not in _NC_CACHE:
        _NC_CACHE[key] = build_nc(cfg, repeat=repeat)
    return _NC_CACHE[key]


def run(inputs, trace=False, repeat=1, **kw):
    from concourse.bass_utils import run_bass_kernel_spmd
    cfg, in_maps, b3 = make_cfg_and_maps(inputs)
    nc = _get_nc(cfg, repeat=repeat)
    res = run_bass_kernel_spmd(nc, in_maps, core_ids=list(range(NCORES)),
                               trace=trace, **kw)
    acc = np.zeros((cfg["C"],), dtype=np.float32)
    for r in res.results:
        acc += r["out"].reshape(-1)
    out = (acc / cfg["N"] + b3.astype(np.float32)).reshape(1, cfg["C"])
    return out, res


def kernel(**inputs) -> np.ndarray:
    out, _ = run(inputs)
    return out


# revision 8
# speedup vs baseline: 1.1213x; 1.0003x over previous
"""GAT (3-layer, PyG GATConv-style) Trainium2 Bass kernel, 8-core SPMD.

Instruction-count-minimized redesign (the axon path serializes at ~60us per
instruction, so wall time ~ total instruction count):

  - Nodes are sorted by in-degree and dealt round-robin to the 8 cores, so
    all cores share one compile-time block structure with near-identical
    per-slot degrees. dst-sharded edge parallelism as before.
  - Per layer, each core computes h_aug rows [h(H*C) | s_src(H)] for its own
    nodes (2 matmuls / 128 nodes), plus a transposed s_dst table [H, slots]
    (5 matmuls), AllGathers the row table to a full DRAM table, then
    aggregates per dst block entirely with wide vector ops:
      * transposed dma_gather pulls the src rows feature-transposed:
        g1[p, k, g, e] = row[idx[e]][128*g + p]  (<=896 idxs per gather)
      * scores e = s_src + s_dst, Prelu(0.2), Exp run on H partitions for a
        whole block per instruction; segment-sum over the fixed per-block
        degree D happens in a single strided tensor_reduce
      * alpha = (p/H) / denom is broadcast to 128 partitions (one SBUF DMA +
        one partition_broadcast), multiplied into the gathered features
        in-place, and a single 5-dim reduce produces the head-averaged
        numerator [C, nd] directly in next-layer-transposed layout.
  - Padding edges point at a sentinel row with s_src = -1e30 (p = 0);
    degree-0 pad slots get one edge to an all-zero neutral row (p > 0,
    h = 0) so denominators stay positive.
  - Layer 3 accumulates the node-sum per block; host sums cores' partials,
    divides by N and adds b3.
"""

import numpy as np
import ml_dtypes

BF16 = ml_dtypes.bfloat16
NCORES = 8

# chunk (= one multi-packet transposed gather) is Dc*nd idxs: <= 2688, %128
def _menu():
    out = []
    for nd in (64, 96, 128, 160, 192, 224, 256, 320, 384, 448, 512):
        for dc in range(2688 // nd, 0, -1):
            if (dc * nd) % 128 == 0:
                out.append((nd, dc))
                break
    return out
MENU = _menu()
CAPE = 5376          # max edges (idx slots) per block, SBUF-driven
SENT_OFF = 0         # sentinel row = NPAD + 0
NEUT_OFF = 1         # neutral row  = NPAD + 1


def _wrap16(idx_flat):
    n = idx_flat.shape[0]
    assert n % 16 == 0
    w = idx_flat.reshape(n // 16, 16).T.astype(np.int16)
    return np.tile(w, (8, 1))


def prep_static(edge_index, N):
    """Degree-sorted node permutation + shared block structure + per-core
    gather index tables."""
    E0 = edge_index.shape[1]
    loops = np.arange(N, dtype=np.int64)
    src = np.concatenate([edge_index[0].astype(np.int64), loops])
    dst = np.concatenate([edge_index[1].astype(np.int64), loops])
    deg = np.bincount(dst, minlength=N)

    order = np.argsort(-deg, kind="stable")
    node_core = np.empty(N, dtype=np.int64)
    node_slot = np.empty(N, dtype=np.int64)
    node_core[order] = np.arange(N) % NCORES
    node_slot[order] = np.arange(N) // NCORES
    NSLOT = (N + NCORES - 1) // NCORES          # 2500
    NPC = ((NSLOT + 127) // 128) * 128          # 2560 slots incl pads
    NPAD = NPC * NCORES

    # per (core, slot) degree; Dmax over cores per slot
    slotdeg = np.zeros((NCORES, NPC), dtype=np.int64)
    slotdeg[node_core, node_slot] = deg
    Dmax = slotdeg.max(axis=0)

    # greedy shared block structure over slots
    blocks = []
    s = 0
    while s < NSLOT:
        Dneed = max(int(Dmax[s]), 1)
        pick = None
        for nd, Dc in sorted(MENU, reverse=True):   # largest nd first
            if nd > NPC - s:
                continue
            K = max(1, -(-Dneed // Dc))
            if K * Dc * nd <= CAPE:
                pick = (nd, Dc, K)
                break
        assert pick is not None, f"no block fits at slot {s}"
        blocks.append((s,) + pick)
        s += pick[0]
    BLKS = tuple(blocks)

    # hf row of original node j (allgather is chunked by CR rows: chunk ck
    # holds rank c's rows [ck*CR,(ck+1)*CR) at ck*CR*NCORES + c*CR + r%CR)
    CR = 512
    hfrow = (node_slot // CR) * (CR * NCORES) + node_core * CR +         (node_slot % CR)

    # per-core per-slot src lists (ordered by slot)
    ecore = node_core[dst]
    eslot = node_slot[dst]
    eorder = np.argsort(ecore * NPC + eslot, kind="stable")
    src_s = src[eorder]
    key_s = (ecore * NPC + eslot)[eorder]
    bounds = np.searchsorted(key_s, np.arange(NCORES * NPC + 1))

    SENT = NPAD + SENT_OFF
    NEUT = NPAD + NEUT_OFF
    idx_cores = []
    for c in range(NCORES):
        cols = []
        for (s0, nd, Dc, K) in BLKS:
            tab = np.full((K * Dc, nd), SENT, dtype=np.int64)
            for n in range(nd):
                g = c * NPC + s0 + n
                lo, hi = int(bounds[g]), int(bounds[g + 1])
                dn = hi - lo
                if dn == 0:
                    tab[0, n] = NEUT
                else:
                    tab[:dn, n] = hfrow[src_s[lo:hi]]
            for k in range(K):
                cols.append(_wrap16(tab[k * Dc:(k + 1) * Dc].reshape(-1)))
        idx_cores.append(np.concatenate(cols, axis=1))
    return BLKS, idx_cores, node_core, node_slot, NPC, NPAD


def prep_values(x, Ws, a_srcs, a_dsts, node_core, node_slot, NPC):
    N, F = x.shape
    xT_cores = np.zeros((NCORES, F, NPC), dtype=np.float32)
    xT_cores[node_core, :, node_slot] = x          # fancy: [N, F] into [c][:,s]
    xT_cores = xT_cores.astype(BF16)

    W_augs, wdsts = [], []
    for W, a_s, a_d in zip(Ws, a_srcs, a_dsts):
        H, Fin, C = W.shape
        RW = H * C + 128
        Wf = np.transpose(W, (1, 0, 2)).reshape(Fin, H * C)
        wsrc = np.einsum("hfc,hc->fh", W, a_s)
        wdst = np.einsum("hfc,hc->fh", W, a_d)
        Wa = np.zeros((Fin, RW), dtype=np.float32)
        Wa[:, :H * C] = Wf
        Wa[:, H * C:H * C + H] = wsrc
        W_augs.append(Wa.astype(BF16))
        wdsts.append(wdst.astype(BF16))
    return xT_cores, W_augs, wdsts


# ----------------------------------------------------------------------------
# Device program
# ----------------------------------------------------------------------------

def build_nc(cfg, repeat=1):
    import concourse.bacc as bacc
    import concourse.mybir as mybir
    import concourse.tile as tile
    from contextlib import ExitStack

    f32 = mybir.dt.float32
    bf16 = mybir.dt.bfloat16
    i16 = mybir.dt.int16
    ALU = mybir.AluOpType
    ACT = mybir.ActivationFunctionType
    AX = mybir.AxisListType

    N = cfg["N"]
    NPC = cfg["NPC"]
    NPAD = NPC * NCORES
    F_IN = cfg["F_IN"]
    C = cfg["C"]
    HS = cfg["HS"]
    BLKS = cfg["BLKS"]
    NB = len(BLKS)
    NL = len(HS)
    RWs = [HS[i] * C + 128 for i in range(NL)]
    FINs = [F_IN] + [C] * (NL - 1)
    SUMI = sum(K * Dc * nd for (_, nd, Dc, K) in BLKS)
    NBA = NPC // 128                      # phase-A 128-node blocks

    nc = bacc.Bacc("TRN2", target_bir_lowering=False, debug=False,
                   num_devices=NCORES)

    xT_d = nc.dram_tensor("xT", [F_IN, NPC], bf16, kind="ExternalInput")
    idx_d = nc.dram_tensor("idx", [128, SUMI // 16], i16, kind="ExternalInput")
    W_d = [nc.dram_tensor(f"w{i+1}", [FINs[i], RWs[i]], bf16,
                          kind="ExternalInput") for i in range(NL)]
    wd_d = [nc.dram_tensor(f"wd{i+1}", [FINs[i], HS[i]], bf16,
                           kind="ExternalInput") for i in range(NL)]
    bb_d = [nc.dram_tensor(f"bb{i+1}", [C, 1], f32, kind="ExternalInput")
            for i in range(NL - 1)]
    out_d = nc.dram_tensor("out", [C, 1], f32, kind="ExternalOutput")

    with tile.TileContext(nc, num_cores=NCORES) as tc, ExitStack() as ctx:
        dram = ctx.enter_context(tc.tile_pool(name="dram", bufs=1, space="DRAM"))
        cpool = ctx.enter_context(tc.tile_pool(name="consts", bufs=1))
        hpool = ctx.enter_context(tc.tile_pool(name="haug", bufs=1))
        gpool = ctx.enter_context(tc.tile_pool(name="gath", bufs=1))
        wpool = ctx.enter_context(tc.tile_pool(name="work", bufs=1))
        apool = ctx.enter_context(tc.tile_pool(name="alpha", bufs=1))
        fpool = ctx.enter_context(tc.tile_pool(name="fin", bufs=1))
        psum = ctx.enter_context(tc.tile_pool(name="ps", bufs=2, space="PSUM"))

        hl = [dram.tile([NPC, RWs[i]], bf16, tag=f"hl{i}", name=f"hl{i}")
              for i in range(NL)]
        hf = [dram.tile([NPAD + 128, RWs[i]], bf16, tag=f"hf{i}",
                        name=f"hf{i}") for i in range(NL)]

        # ---- constants ----
        xT_sb = cpool.tile([F_IN, NPC], bf16, tag="xT")
        nc.sync.dma_start(xT_sb[:], xT_d[:, :])
        idx_sb = cpool.tile([128, SUMI // 16], i16, tag="idx")
        nc.sync.dma_start(idx_sb[:], idx_d[:, :])
        W_sb, wd_sb, bb_sb = [], [], []
        for i in range(NL):
            w = cpool.tile([FINs[i], RWs[i]], bf16, tag=f"w{i}", name=f"w{i}")
            nc.sync.dma_start(w[:], W_d[i][:, :])
            W_sb.append(w)
            wd = cpool.tile([FINs[i], HS[i]], bf16, tag=f"wd{i}", name=f"wd{i}")
            nc.sync.dma_start(wd[:], wd_d[i][:, :])
            wd_sb.append(wd)
        for i in range(NL - 1):
            b = cpool.tile([C, 1], f32, tag=f"bb{i}", name=f"bb{i}")
            nc.sync.dma_start(b[:], bb_d[i][:, :])
            bb_sb.append(b)

        # sentinel (s_src = -1e30) + neutral (all zero) rows per layer table
        for L in range(NL):
            HC = HS[L] * C
            srow = hpool.tile([1, 2, RWs[L]], bf16, tag="srow", name="srow")
            nc.vector.memset(srow[:], 0.0)
            nc.vector.memset(srow[:, 0, HC:HC + HS[L]], -1e30)
            nc.sync.dma_start(hf[L][NPAD:NPAD + 2, :], srow[:])

        # next-layer transposed features (phase C writes, phase A reads)
        x2T = [cpool.tile([C, NPC], bf16, tag=f"x2T{i}", name=f"x2T{i}")
               for i in range(NL - 1)]
        for t in x2T:
            nc.vector.memset(t[:], 0.0)

        nout = cpool.tile([C, NPC], f32, tag="nout")
        nc.vector.memset(nout[:], 0.0)

        # hoisted num_idxs registers (avoid one RegisterMove per gather)
        cnds = sorted({Dc * nd for (_, nd, Dc, K) in BLKS})
        cnd_reg = {v: nc.gpsimd.to_reg(v) for v in cnds}

        for _rep in range(repeat):
            for L in range(NL):
                H = HS[L]
                RW = RWs[L]
                G = RW // 128
                HC = H * C
                xin = xT_sb if L == 0 else x2T[L - 1]

                # ---- phase A: h_aug rows for own slots ----
                HB = NBA // 2
                for h0 in (0, HB):
                 hs = hpool.tile([128, HB, RW], bf16, tag="hs", name="hs")
                 for nb in range(h0, h0 + HB):
                    lhs = xin[:, nb * 128:(nb + 1) * 128]
                    if RW > 512:
                        p1 = psum.tile([128, 640], f32, tag="pA", bufs=1)
                        nc.tensor.matmul(p1[:, 0:512], lhs, W_sb[L][:, 0:512],
                                         start=True, stop=True)
                        nc.tensor.matmul(p1[:, 512:RW], lhs, W_sb[L][:, 512:RW],
                                         start=True, stop=True)
                        nc.scalar.copy(hs[:, nb - h0, 0:RW], p1[:, 0:RW])
                    elif nb % 2 == 0:
                        # pair two 256-wide blocks in one PSUM bank + one copy
                        p1 = psum.tile([128, 512], f32, tag="pA", bufs=1)
                        nc.tensor.matmul(p1[:, 0:RW], lhs, W_sb[L][:, 0:RW],
                                         start=True, stop=True)
                        nc.tensor.matmul(p1[:, RW:2 * RW],
                                         xin[:, (nb + 1) * 128:(nb + 2) * 128],
                                         W_sb[L][:, 0:RW],
                                         start=True, stop=True)
                        nc.scalar.copy(hs[:, nb - h0:nb - h0 + 2, :], p1[:])
                 orows = hl[L][h0 * 128:(h0 + HB) * 128, :].rearrange(
                     "(j p) w -> p j w", j=HB)
                 nc.sync.dma_start(orows, hs[:])

                # ---- transposed s_dst for own slots: [H, NPC] ----
                sdT = wpool.tile([HS[L], NPC], bf16, tag="sdT", name="sdT")
                ps = psum.tile([HS[L], NPC], f32, tag="pS", bufs=1)
                for j in range(NPC // 512):
                    nc.tensor.matmul(ps[:, j * 512:(j + 1) * 512], wd_sb[L],
                                     xin[:, j * 512:(j + 1) * 512],
                                     start=True, stop=True)
                nc.scalar.copy(sdT[:], ps[:])

                # ---- allgather ----
                CR = 512
                for ck in range(NPC // CR):
                    nc.gpsimd.collective_compute(
                        "AllGather", mybir.AluOpType.bypass,
                        replica_groups=[list(range(NCORES))],
                        ins=[hl[L][ck * CR:(ck + 1) * CR, :].opt()],
                        outs=[hf[L][ck * CR * NCORES:(ck + 1) * CR * NCORES,
                                    :].opt()],
                    )

                # ---- phase C ----
                col = 0
                bi = 0
                while bi < NB:
                    (s0, nd, Dc, K) = BLKS[bi]
                    CND = Dc * nd
                    # H==1: pair adjacent identical blocks (alpha needs no
                    # 1/H scale, so 4-dim tensor_tensor normalizes both)
                    M = 2 if (H == 1 and bi + 1 < NB and
                              BLKS[bi + 1] == (s0 + nd, nd, Dc, K)) else 1
                    MK = M * K
                    g1 = gpool.tile([128, MK, G, CND], bf16, tag="g1")
                    for k in range(MK):
                        nc.gpsimd.dma_gather(
                            g1[:, k, :, :], hf[L][:, :],
                            idx_sb[:, col:col + CND // 16],
                            CND, cnd_reg[CND], RW, transpose=True,
                            single_packet=False)
                        col += CND // 16

                    if H > 1:
                        # scores -> p (in place), on H partitions
                        e = wpool.tile([H, K, Dc, nd], f32, tag="e")
                        ssrc = g1[0:H, :, G - 1, :].rearrange(
                            "p k (d n) -> p k d n", d=Dc)
                        sd_v = sdT[:, s0:s0 + nd].unsqueeze(1).unsqueeze(1) \
                            .broadcast_to((H, K, Dc, nd))
                        nc.vector.tensor_tensor(e[:], ssrc, sd_v, ALU.add)
                        e_3 = e[:].rearrange("p k d n -> p (k d) n")
                        nc.vector.scalar_tensor_tensor(e_3, e_3, 0.2, e_3,
                                                       op0=ALU.mult,
                                                       op1=ALU.max)
                        nc.scalar.activation(e[:], e[:], ACT.Exp)

                        den = wpool.tile([H, nd], f32, tag="den")
                        nc.vector.tensor_reduce(den[:],
                                                e[:].transpose([0, 3, 1, 2]),
                                                AX.XY, ALU.add)
                        rc = wpool.tile([H, nd], f32, tag="rc")
                        nc.vector.reciprocal(rc[:], den[:])
                        al = wpool.tile([H, K, Dc, nd], bf16, tag="al")
                        rc_v = rc[:].unsqueeze(1).broadcast_to((H, K * Dc, nd))
                        e_f = e[:].rearrange("p k d n -> p (k d) n")
                        al_f = al[:].rearrange("p k d n -> p (k d) n")
                        nc.vector.scalar_tensor_tensor(al_f, e_f, 1.0 / H,
                                                       rc_v, op0=ALU.mult,
                                                       op1=ALU.mult)

                        aa = apool.tile([128, H, K * CND], bf16, tag="aa")
                        adr = dram.tile([H, K * CND], bf16, tag="adr",
                                        name="adr")
                        nc.gpsimd.dma_start(adr[:], al[:])
                        nc.gpsimd.dma_start(
                            aa[:], adr[:].unsqueeze(0).broadcast_to(
                                (128, H, K * CND)))

                        g1f = g1[:, :, 0:H, :]
                        aa_v = aa[:].rearrange("p h (k c) -> p k h c", k=K)
                        nc.vector.tensor_tensor(g1f, g1f, aa_v, ALU.mult)

                        num_t = fpool.tile([C, nd], f32, tag="num",
                                           name="num_t")
                        nv = g1[:, :, 0:H, :].rearrange(
                            "p k h (d n) -> p n h k d", d=Dc)
                        nc.vector.tensor_reduce(num_t[:], nv, AX.XYZ, ALU.add)
                        nc.vector.tensor_scalar(x2T[L][:, s0:s0 + nd],
                                                num_t[:], bb_sb[L][:], 0.0,
                                                op0=ALU.add, op1=ALU.max)
                    else:
                        # paired single-head path, scores in bf16
                        e = wpool.tile([1, MK, Dc, nd], bf16, tag="e",
                                       name="e3")
                        for m in range(M):
                            ssrc_m = g1[0:1, m * K:(m + 1) * K, G - 1, :] \
                                .rearrange("p k (d n) -> p k d n", d=Dc)
                            sd_m = sdT[0:1, s0 + m * nd:s0 + (m + 1) * nd] \
                                .unsqueeze(1).unsqueeze(1) \
                                .broadcast_to((1, K, Dc, nd))
                            nc.vector.tensor_tensor(
                                e[:, m * K:(m + 1) * K, :, :], ssrc_m, sd_m,
                                ALU.add)
                        e_3 = e[:].rearrange("p k d n -> p (k d) n")
                        nc.vector.scalar_tensor_tensor(e_3, e_3, 0.2, e_3,
                                                       op0=ALU.mult,
                                                       op1=ALU.max)
                        nc.scalar.activation(e[:], e[:], ACT.Exp)

                        den = wpool.tile([1, M, nd], f32, tag="den",
                                         name="den3")
                        dv = e[:].rearrange("p (m k) d n -> p m n (k d)", m=M)
                        nc.vector.tensor_reduce(den[:], dv, AX.X, ALU.add)
                        rc = wpool.tile([1, M, nd], f32, tag="rc", name="rc3")
                        nc.vector.reciprocal(rc[:], den[:])
                        al = wpool.tile([1, MK, Dc, nd], bf16, tag="al")
                        al_v = al[:].rearrange("p (m k) d n -> p m (k d) n",
                                               m=M)
                        e_v2 = e[:].rearrange("p (m k) d n -> p m (k d) n",
                                              m=M)
                        rc_v = rc[:].unsqueeze(2).broadcast_to(
                            (1, M, K * Dc, nd))
                        nc.vector.tensor_tensor(al_v, e_v2, rc_v, ALU.mult)

                        aa = apool.tile([128, MK * CND], bf16, tag="aa")
                        nc.gpsimd.partition_broadcast(aa[:],
                                                      al[0:1, :, :, :])
                        g1f = g1[:, :, 0, :]
                        aa_v = aa[:].rearrange("p (k c) -> p k c", k=MK)
                        nc.vector.tensor_tensor(g1f, g1f, aa_v, ALU.mult)

                        nv = g1[:, :, 0, :].rearrange(
                            "p (m k) (d n) -> p m n k d", m=M, d=Dc)
                        nc.vector.tensor_reduce(nout[:, s0:s0 + M * nd], nv,
                                                AX.XY, ALU.add)
                    bi += M

        fo = fpool.tile([C, 1], f32, tag="fo")
        nc.vector.tensor_reduce(fo[:], nout[:], AX.X, ALU.add)
        nc.sync.dma_start(out_d[:, :], fo[:])

    nc.compile()
    return nc


# ----------------------------------------------------------------------------
# Entry points
# ----------------------------------------------------------------------------

def make_cfg_and_maps(inputs):
    x = np.asarray(inputs["x"])
    edge_index = np.asarray(inputs["edge_index"])
    N, F_IN = x.shape
    Ws = [np.asarray(inputs[f"W{i}"]) for i in (1, 2, 3)]
    a_srcs = [np.asarray(inputs[f"as{i}"]) for i in (1, 2, 3)]
    a_dsts = [np.asarray(inputs[f"ad{i}"]) for i in (1, 2, 3)]
    bs = [np.asarray(inputs[f"b{i}"]) for i in (1, 2, 3)]
    HS = tuple(W.shape[0] for W in Ws)
    C = Ws[0].shape[2]

    BLKS, idx_cores, node_core, node_slot, NPC, NPAD = \
        prep_static(edge_index, N)
    xT_cores, W_augs, wdsts = prep_values(
        x, Ws, a_srcs, a_dsts, node_core, node_slot, NPC)

    cfg = dict(N=N, NPC=NPC, F_IN=F_IN, C=C, HS=HS, BLKS=BLKS)
    in_maps = []
    for c in range(NCORES):
        m = {
            "xT": np.ascontiguousarray(xT_cores[c]),
            "idx": idx_cores[c],
        }
        for i in range(3):
            m[f"w{i+1}"] = W_augs[i]
            m[f"wd{i+1}"] = wdsts[i]
        for i in range(2):
            m[f"bb{i+1}"] = bs[i].astype(np.float32).reshape(C, 1)
        in_maps.append(m)
    return cfg, in_maps, bs[2]


_NC_CACHE = {}


def _get_nc(cfg, repeat=1):
    key = (repeat, cfg["N"], cfg["NPC"], cfg["F_IN"], cfg["C"], cfg["HS"],
           cfg["BLKS"])
    if key not in _NC_CACHE:
        _NC_CACHE[key] = build_nc(cfg, repeat=repeat)
    return _NC_CACHE[key]


def run(inputs, trace=False, repeat=1, **kw):
    from concourse.bass_utils import run_bass_kernel_spmd
    cfg, in_maps, b3 = make_cfg_and_maps(inputs)
    nc = _get_nc(cfg, repeat=repeat)
    res = run_bass_kernel_spmd(nc, in_maps, core_ids=list(range(NCORES)),
                               trace=trace, **kw)
    acc = np.zeros((cfg["C"],), dtype=np.float32)
    for r in res.results:
        acc += r["out"].reshape(-1)
    out = (acc / cfg["N"] + b3.astype(np.float32)).reshape(1, cfg["C"])
    return out, res


def kernel(**inputs) -> np.ndarray:
    out, _ = run(inputs)
    return out
